# revision 47
# baseline (speedup 1.0000x reference)
"""EquiformerV2 (2-layer) Bass/Tile kernel for 8 trn2 NeuronCores — v3.

Sharding: dst-node-range parallel (core c owns nodes [256c, 256c+256) and the
edges terminating there). Per attention: y_s/y_t computed locally in bf16, one
AllGather of y_s, then per 128-edge tile gathered messages feed transposes,
values, logits and a one-hot-matmul scatter.

v3 vs v2: packed weight loads (2 DMAs), radial MLPs batched to 512-col
streams with bias folded into the activation, rad modulation fused into the
transpose-PSUM drain (h-major rad), software-pipelined edge loop (T(i+1)
issued before V(i)), engine-partitioned drains (vector=T, scalar=V), FFN gate
reads PSUM directly, Rsqrt-based norms, pool-first latent epilogue.
"""
import math
from contextlib import ExitStack

import numpy as np

import concourse.bass as bass
import concourse.bacc as bacc
import concourse.mybir as mybir
import concourse.tile as tile
from concourse.bass_utils import run_bass_kernel_spmd
from concourse.masks import make_identity

F32 = mybir.dt.float32
BF = mybir.dt.bfloat16
I32 = mybir.dt.int32
AF = mybir.ActivationFunctionType
ALU = mybir.AluOpType
AX = mybir.AxisListType
BF_NP = mybir.dt.np(BF)

NCORES = 8
L_MAX, M_MAX = 6, 2
NC49 = (L_MAX + 1) ** 2
C = 128
H = 128
HEADS, VPH = 8, 16
FFN = 512
NB = 600
N, E, G = 2048, 12288, 16
NP = N // NCORES
AVG_DEG = 3.0
CUTOFF = 5.0
DISC_LO, DISC_HI = -3.26267, 3.295396
EPS = 1e-6

LBLK = [(l * l, 2 * l + 1) for l in range(L_MAX + 1)]
RBLK = []
_r = 0
for _l in range(L_MAX + 1):
    _cnt = min(2 * _l + 1, 2 * M_MAX + 1)
    RBLK.append((_r, _l * _l + _l - min(_l, M_MAX), _cnt))
    _r += _cnt
NR = _r                   # 29
W29 = NR * 128
W49 = NC49 * 128
RCH8 = [(r0, min(8, NR - r0)) for r0 in range(0, NR, 8)]
RCH4 = [(r0, min(4, NR - r0)) for r0 in range(0, NR, 4)]

_off_np = np.linspace(0.0, CUTOFF, NB).astype(np.float32)
GCOEF = float(-0.5 / (2.0 * (_off_np[1] - _off_np[0])) ** 2)
_mv_np = np.array([m for l in range(L_MAX + 1) for m in range(-l, l + 1)])
_deg_np = np.array([l for l in range(L_MAX + 1) for m in range(-l, l + 1)])
RESTRICT_NP = np.nonzero(np.abs(_mv_np) <= M_MAX)[0]

# bf16 weight pack layout: (name, cols). [120-row blocks live in rows 0:120.]
PACKB = [
    ("w_st", 3 * 256), ("w_v", 3 * 128), ("w_p", 3 * 128),
    ("rad_w2", 3 * 128), ("Hsel", 8), ("ffn_w1", 2 * FFN), ("ffn_w2", 8 * 128),
    ("degw2", 128), ("degw3", 7 * C), ("degw1c", 5 * 128), ("radw1c", 15 * 128),
]
PBOFF = {}
_o = 0
for _n, _w in PACKB:
    PBOFF[_n] = _o
    _o += _w
PBW = _o
# f32 pack: nwT 35 | avecC 3 | offc 5 (rows 0:120) | degb1 1 | degb2 1 | radb1 3
PFW = 48


def real_sph_harm_np(vec):
    r = np.linalg.norm(vec, axis=-1, keepdims=True)
    u = vec / np.maximum(r, 1e-8)
    x, y, z = u[:, 0], u[:, 1], u[:, 2]
    ct = np.clip(z, -1.0, 1.0)
    st = np.sqrt(np.clip(1.0 - ct * ct, 1e-12, 1.0))
    phi = np.arctan2(y, x)
    P = {(0, 0): np.ones_like(ct)}
    for m in range(1, L_MAX + 1):
        P[(m, m)] = -(2 * m - 1) * st * P[(m - 1, m - 1)]
    for m in range(0, L_MAX):
        P[(m + 1, m)] = (2 * m + 1) * ct * P[(m, m)]
    for m in range(0, L_MAX + 1):
        for l in range(m + 2, L_MAX + 1):
            P[(l, m)] = ((2 * l - 1) * ct * P[(l - 1, m)] - (l + m - 1) * P[(l - 2, m)]) / (l - m)
    cols = []
    for l in range(L_MAX + 1):
        for m in range(-l, l + 1):
            am = abs(m)
            nrm = math.sqrt((2 * l + 1) / (4 * math.pi) * math.factorial(l - am) / math.factorial(l + am))
            if m == 0:
                cols.append(nrm * P[(l, 0)])
            elif m > 0:
                cols.append(math.sqrt(2.0) * nrm * P[(l, m)] * np.cos(m * phi))
            else:
                cols.append(math.sqrt(2.0) * nrm * P[(l, am)] * np.sin(am * phi))
    return np.stack(cols, axis=-1).astype(np.float32)


def host_prep(inputs):
    f = lambda k: np.asarray(inputs[k], np.float32)
    pos = f("pos")
    edge_vec = f("edge_vec")
    edge_index = np.asarray(inputs["edge_index"]).astype(np.int64)
    batch = np.asarray(inputs["batch"]).astype(np.int64)

    src, dst = edge_index[0], edge_index[1]
    d_all = np.linalg.norm(edge_vec, axis=-1).astype(np.float32)
    Y_all = (real_sph_harm_np(edge_vec) / np.float32(AVG_DEG)).astype(np.float32)

    t = np.clip(np.round((pos - DISC_LO) / (DISC_HI - DISC_LO) * 128.0 - 0.5), 0, 127).astype(np.int64)
    et_ = f("embed_table")
    emb = (et_[t[:, 0]] + et_[t[:, 1]] + et_[t[:, 2]]).astype(np.float32)

    core_of = dst // NP
    grp_of = (dst % NP) // 128
    lists = [[np.nonzero((core_of == c) & (grp_of == g))[0] for g in range(2)] for c in range(NCORES)]
    TG = max(1, (max(len(lists[c][g]) for c in range(NCORES) for g in range(2)) + 127) // 128)
    NT = 2 * TG
    EP = NT * 128

    cnt = np.bincount(batch, minlength=G).astype(np.float32)
    inv_cnt = (1.0 / np.maximum(cnt, 1.0)).astype(np.float32)

    # ---- f32 pack ----
    nws = [f("attn_norm_w")[0], f("ffn_norm_w")[0], f("attn_norm_w")[1], f("ffn_norm_w")[1], f("final_norm_w")]
    packF = np.zeros((128, PFW), np.float32)
    packF[:, 0:35] = np.concatenate([w.T for w in nws], axis=1)
    packF[:, 35:38] = np.stack([f("alpha_vec")[0].reshape(-1), f("alpha_vec")[1].reshape(-1),
                                f("lat_alpha").reshape(-1)], axis=1)
    packF[0:120, 38:43] = np.ascontiguousarray(_off_np.reshape(5, 120).T)
    packF[:, 43] = f("deg_b1")
    packF[:, 44] = f("deg_b2")
    packF[:, 45] = f("rad_b1")[0]
    packF[:, 46] = f("rad_b1")[1]
    packF[:, 47] = f("lat_rad_b1")

    # ---- bf16 pack ----
    def stack_lat(key, lat_key):
        return np.concatenate([f(key)[0], f(key)[1], f(lat_key)], axis=1)  # [128, 3*128]

    packB = np.zeros((128, PBW), np.float32)

    def put(name, arr, rows=128):
        o = PBOFF[name]
        packB[0:rows, o:o + arr.shape[1]] = arr

    ws_ = stack_lat("w_src", "lat_w_src")     # [128, 3*128]
    wt_ = stack_lat("w_tgt", "lat_w_tgt")
    wst = np.zeros((128, 3 * 256), np.float32)
    for a_ in range(3):
        wst[:, a_ * 256:a_ * 256 + 128] = ws_[:, a_ * 128:(a_ + 1) * 128]
        wst[:, a_ * 256 + 128:a_ * 256 + 256] = wt_[:, a_ * 128:(a_ + 1) * 128]
    put("w_st", wst)
    put("w_v", stack_lat("w_val", "lat_w_val"))
    put("w_p", stack_lat("w_proj", "lat_w_proj"))
    put("rad_w2", stack_lat("rad_w2", "lat_rad_w2"))
    Hsel = np.zeros((128, HEADS), np.float32)
    Hsel[np.arange(128), np.arange(128) // VPH] = 1.0
    put("Hsel", Hsel)
    put("ffn_w1", np.concatenate([f("ffn_w1")[0], f("ffn_w1")[1]], axis=1))
    w2 = np.concatenate([f("ffn_w2")[0], f("ffn_w2")[1]], axis=0)  # [1024, 128]
    put("ffn_w2", w2.reshape(8, 128, 128).transpose(1, 0, 2).reshape(128, 8 * 128))
    put("degw2", f("deg_w2"))
    put("degw3", f("deg_w3"))
    put("degw1c", f("deg_w1").reshape(5, 120, C).transpose(1, 0, 2).reshape(120, 5 * C), rows=120)
    rw1 = np.stack([f("rad_w1")[0], f("rad_w1")[1], f("lat_rad_w1")], axis=0)  # [3, 600, H]
    put("radw1c", rw1.reshape(3, 5, 120, H).transpose(2, 0, 1, 3).reshape(120, 15 * H), rows=120)

    shared = {"packF": packF, "packB": packB.astype(BF_NP)}

    in_maps = []
    for c in range(NCORES):
        srcg = np.zeros((EP,), np.int64)
        dstg = np.zeros((EP,), np.int64)
        dstS = np.full((EP,), 30000, np.int64)
        d_row = np.zeros((1, EP), np.float32)
        Yc = np.zeros((EP, NC49), np.float32)
        for g in range(2):
            idx = lists[c][g]
            o = g * TG * 128
            n = len(idx)
            srcg[o:o + n] = src[idx]
            dstg[o:o + n] = dst[idx] - c * NP
            dstS[o:o + n] = dst[idx] - c * NP
            d_row[0, o:o + n] = d_all[idx]
            Yc[o:o + n] = Y_all[idx]
        embT = np.ascontiguousarray(emb[c * NP:(c + 1) * NP].T)
        PT = np.zeros((NP, G), np.float32)
        nloc = np.arange(c * NP, (c + 1) * NP)
        PT[np.arange(NP), batch[nloc]] = inv_cnt[batch[nloc]]
        YtT = np.ascontiguousarray(Yc.reshape(NT, 128, NC49).transpose(1, 0, 2).reshape(128, NT * NC49))
        # ys_full row layout after split AG: rows [0:1024) hold every core's
        # group-0 nodes (row = core*128 + n%128), rows [1024:2048) group-1.
        srcr = (srcg // NP) * 128 + (srcg % 128) + 1024 * ((srcg % NP) // 128)
        idxs = np.zeros((128, 3 * NT), np.int32)
        idxs[:, 0:NT] = srcr.reshape(NT, 128).T
        idxs[:, NT:2 * NT] = dstg.reshape(NT, 128).T
        idxs[:, 2 * NT:3 * NT] = dstS.reshape(NT, 128).T
        m = dict(shared)
        m.update({"embT": embT, "d_row": d_row, "Yt": YtT.astype(BF_NP),
                  "idxs": idxs, "PT": PT.astype(BF_NP)})
        in_maps.append(m)
    return {"TG": TG, "NT": NT, "EP": EP}, in_maps


def _chunks(total, step=512):
    o = 0
    while o < total:
        yield o, min(step, total - o)
        o += step


def build_program(meta, debug=(), ablate=frozenset()):
    TG, NT, EP = meta["TG"], meta["NT"], meta["EP"]
    nc = bacc.Bacc("TRN2", target_bir_lowering=False, debug=False, num_devices=NCORES)

    def din(name, shape, dt=F32):
        return nc.dram_tensor(name, shape, dt, kind="ExternalInput")

    packF_d = din("packF", [128, PFW])
    packB_d = din("packB", [128, PBW], BF)
    embT_d = din("embT", [128, NP])
    d_row_d = din("d_row", [1, EP])
    Yt_d = din("Yt", [128, NT * NC49], BF)
    idxs_d = din("idxs", [128, 3 * NT], I32)
    PT_d = din("PT", [NP, G], BF)

    pooled_d = nc.dram_tensor("pooled", [128, NR * G], F32, kind="ExternalOutput")
    dbg_d = {name: nc.dram_tensor("dbg_" + name, list(shape), F32, kind="ExternalOutput")
             for name, shape in debug}

    ys_loc = nc.dram_tensor("ys_loc", [NP, W29], BF)
    yt_loc = nc.dram_tensor("yt_loc", [NP, W29], BF)
    ys_full = nc.dram_tensor("ys_full", [N, W29], BF, addr_space="Shared")
    RG = [list(range(NCORES))]

    with tile.TileContext(nc) as tc, ExitStack() as es:
        per = es.enter_context(tc.tile_pool(name="persist", bufs=1))

        def dbg(name, ap):
            if name in dbg_d:
                if ap.dtype != F32:
                    nc.gpsimd.dma_start(dbg_d[name][:], ap)
                else:
                    nc.sync.dma_start(dbg_d[name][:], ap)

        # ---- persistent tiles ----
        PB = per.tile([128, PBW], BF, tag="PB")
        nc.sync.dma_start(PB[:], packB_d[:])
        PF = per.tile([128, PFW], F32, tag="PF")
        nc.scalar.dma_start(PF[:], packF_d[:])
        idxs = per.tile([128, 3 * NT], I32, tag="idxs")
        nc.gpsimd.dma_start(idxs[:], idxs_d[:])
        PT = [per.tile([128, G], BF, tag=f"PT{g}", name=f"PT{g}") for g in range(2)]
        for g in range(2):
            nc.scalar.dma_start(PT[g][:], PT_d[g * 128:(g + 1) * 128, :])

        def wST(a):
            return PB[:, PBOFF["w_st"] + a * 256:PBOFF["w_st"] + (a + 1) * 256]

        def wV(a):
            return PB[:, PBOFF["w_v"] + a * 128:PBOFF["w_v"] + (a + 1) * 128]

        def wP(a):
            return PB[:, PBOFF["w_p"] + a * 128:PBOFF["w_p"] + (a + 1) * 128]

        def rW2(a):
            return PB[:, PBOFF["rad_w2"] + a * 128:PBOFF["rad_w2"] + (a + 1) * 128]

        def rW1(a, ci):
            o = PBOFF["radw1c"] + (a * 5 + ci) * 128
            return PB[0:120, o:o + 128]

        def fW1(i, fc):
            o = PBOFF["ffn_w1"] + i * FFN + fc * 128
            return PB[:, o:o + 128]

        def fW2(i, fc):
            o = PBOFF["ffn_w2"] + (i * 4 + fc) * 128
            return PB[:, o:o + 128]

        Hsel = PB[:, PBOFF["Hsel"]:PBOFF["Hsel"] + HEADS]
        degw2 = PB[:, PBOFF["degw2"]:PBOFF["degw2"] + 128]
        degw3 = PB[:, PBOFF["degw3"]:PBOFF["degw3"] + 7 * C]

        def dW1(ci):
            o = PBOFF["degw1c"] + ci * 128
            return PB[0:120, o:o + 128]

        nwT = PF[:, 0:35]
        avecC = PF[:, 35:38]
        offc = PF[0:120, 38:43]
        degb1 = PF[:, 43:44]
        degb2 = PF[:, 44:45]

        def radb1(a):
            return PF[:, 45 + a:46 + a]

        ident = per.tile([128, 128], F32, tag="ident")
        make_identity(nc, ident[:])
        ident_b = per.tile([128, 128], BF, tag="identb")
        nc.vector.tensor_copy(ident_b[:], ident[:])
        ones1f = per.tile([1, 128], F32, tag="ones1f")
        nc.vector.memset(ones1f[:], 1.0)
        ones128b = per.tile([128, 128], BF, tag="ones128")
        nc.vector.memset(ones128b[:], 1.0)
        epsc = per.tile([128, 1], F32, tag="epsc")
        nc.vector.memset(epsc[:], EPS)

        xT = [per.tile([128, W49], F32, tag=f"xT{g}", name=f"xT{g}") for g in range(2)]
        S_all = per.tile([128, NT * 128], BF, tag="S_all")
        distT = per.tile([120, 5 * EP], BF, tag="distT")

        copy_rr = [nc.scalar, nc.vector]

        def copy_eng(i, out_ap, in_ap):
            e = copy_rr[i % len(copy_rr)]
            if e is nc.scalar:
                e.copy(out_ap, in_ap)
            else:
                e.tensor_copy(out_ap, in_ap)

        def tt_eng(i):
            return [nc.vector, nc.gpsimd][i % 2]

        # ---------- rms norm split: stats-pre (sq/red), stats-fin (ms/rsqrt) ----
        def stats_pre(g, sq_t, red):
            """red[c,(l n)] = bf16 partials of sum_k x^2; sq_t >=3136-col scratch."""
            QN = 1568
            with nc.allow_low_precision(reason="bf16 ms-reduce, 0.4% on rms"):
                for qi in range(4):
                    sqq = sq_t[:, (qi % 2) * QN:(qi % 2 + 1) * QN]
                    xq = xT[g][:, qi * QN:(qi + 1) * QN]
                    [nc.vector, nc.gpsimd][qi % 2].tensor_tensor(sqq, xq, xq, op=ALU.mult)
                    for l in range(L_MAX + 1):
                        ks, kc = LBLK[l]
                        nc.vector.tensor_reduce(
                            red[:, l * 128 + qi * 32:l * 128 + (qi + 1) * 32],
                            sqq.rearrange("p (n k) -> p n k", k=NC49)[:, :, ks:ks + kc],
                            axis=AX.X, op=ALU.add)

        def stats_fin(key, nidx, g, red, psp, psum_tag):
            scl = per.tile([128, 896], F32, tag="scl", bufs=2, name=f"scl_{key}_{g}")
            msA = psp.tile([128, 512], F32, tag=psum_tag, space="PSUM", bufs=2)
            nc.tensor.matmul(msA[:, 0:512], lhsT=ones128b[:], rhs=red[:, 0:512],
                             start=True, stop=True)
            msB = psp.tile([128, 512], F32, tag=psum_tag, space="PSUM", bufs=2)
            nc.tensor.matmul(msB[:, 0:384], lhsT=ones128b[:], rhs=red[:, 512:896],
                             start=True, stop=True)
            for l in range(L_MAX + 1):
                msl = msA[:, l * 128:(l + 1) * 128] if l < 4 else msB[:, (l - 4) * 128:(l - 3) * 128]
                nc.scalar.activation(scl[:, l * 128:(l + 1) * 128], msl, AF.Ln,
                                     bias=epsc[:], scale=float(1.0 / ((2 * l + 1) * C)))
            nc.scalar.activation(scl[:], scl[:], AF.Exp, scale=-0.5)
            for l in range(L_MAX + 1):
                sl = scl[:, l * 128:(l + 1) * 128]
                nc.vector.tensor_scalar(sl, sl, nwT[:, nidx * 7 + l:nidx * 7 + l + 1],
                                        None, op0=ALU.mult)
            pend_scl[(key, g)] = scl

        def rms_apply(restricted, out_tile, g, scl):
            stride = NR if restricted else NC49
            blocks = RBLK if restricted else [(ks, ks, kc) for (ks, kc) in LBLK]
            for l, (os_, ks, cnt) in enumerate(blocks):
                if restricted:
                    ov = out_tile[:].rearrange("p (n k) -> p n k", k=stride)[:, :, os_:os_ + cnt]
                    xv = xT[g][:].rearrange("p (n k) -> p n k", k=NC49)[:, :, ks:ks + cnt]
                    iv = scl[:, l * 128:(l + 1) * 128].rearrange("p n -> p n ()") \
                        .to_broadcast([128, 128, cnt])
                else:
                    ov = out_tile[:].rearrange("p (q k n) -> p q n k", q=4, n=32)[
                        :, :, :, os_:os_ + cnt]
                    xv = xT[g][:].rearrange("p (q n k) -> p q n k", q=4, k=NC49)[
                        :, :, :, ks:ks + cnt]
                    iv = scl[:, l * 128:(l + 1) * 128].rearrange("p (q n) -> p q n ()", q=4) \
                        .to_broadcast([128, 4, 32, cnt])
                tt_eng(l).tensor_tensor(ov, xv, iv, op=ALU.mult)

        pend_scl = {}
        pend_red = {}

        # ---------- combined ys||yt rows for one group: 29 mm + drains + DMAs ----
        def yrows_st(hr, a, g, ypp, yss, dbg_pref=None):
            hv = hr[:].rearrange("p (n k) -> p k n", k=NR)
            ysrow = yss.tile([128, W29], BF, tag="ysrow", bufs=1)
            ytrow = yss.tile([128, W29], BF, tag="ytrow", bufs=1)
            for bi, (r0, nr) in enumerate(RCH4):
                yp = ypp.tile([128, 1024], F32, tag="yp", space="PSUM", bufs=2)
                for j in range(nr):
                    nc.tensor.matmul(yp[:, j * 256:(j + 1) * 256],
                                     lhsT=hv[:, r0 + j, :], rhs=wST(a),
                                     start=True, stop=True)
                copy_eng(bi, ysrow[:, r0 * 128:(r0 + nr) * 128]
                             .rearrange("p (j c) -> p j c", c=128),
                         yp[:].rearrange("p (j c) -> p j c", c=256)[:, 0:nr, 0:128])
                copy_eng(bi + 1, ytrow[:, r0 * 128:(r0 + nr) * 128]
                             .rearrange("p (j c) -> p j c", c=128),
                         yp[:].rearrange("p (j c) -> p j c", c=256)[:, 0:nr, 128:256])
            if dbg_pref:
                dbg("ysr0", ysrow[:])
                dbg("ytr0", ytrow[:])
            nc.sync.dma_start(ys_loc[g * 128:(g + 1) * 128, :], ysrow[:])
            if "ag" not in ablate:
                nc.gpsimd.collective_compute(
                    "AllGather", ALU.bypass, replica_groups=RG,
                    ins=[ys_loc[g * 128:(g + 1) * 128, :]],
                    outs=[ys_full[g * 1024:(g + 1) * 1024, :]])
            nc.scalar.dma_start(yt_loc[g * 128:(g + 1) * 128, :], ytrow[:])

        def ys_rows(a, g):
            """apply norm + ys/yt rows + AG part for group g (stats precomputed)."""
            scl = pend_scl.pop(("att%d" % a, g))
            with tc.tile_pool(name=f"ys{a}g{g}", bufs=1) as sbp:
                hrT = sbp.tile([128, W29], BF, tag="hrT")
                rms_apply(True, hrT, g, scl)
                with tc.tile_pool(name=f"ys{a}g{g}p", bufs=1, space="PSUM") as ypp:
                    yrows_st(hrT, a, g, ypp, sbp,
                             dbg_pref=(a == 0 and g == 0))

        # ---------- radial MLP for attention a (h-major output) ----------
        def rad_mlp(a, radT, rs, rp):
            for o, w in _chunks(EP, 512):
                ps = rp.tile([128, 512], F32, tag="rmlp1", space="PSUM", bufs=2)
                for ci in range(5):
                    nc.tensor.matmul(ps[:, 0:w], lhsT=rW1(a, ci),
                                     rhs=distT[:, ci * EP + o:ci * EP + o + w],
                                     start=(ci == 0), stop=(ci == 4))
                s1 = rs.tile([128, 512], BF, tag="rm_s1")
                nc.scalar.activation(s1[:, 0:w], ps[:, 0:w], AF.Silu, bias=radb1(a))
                ps2 = rp.tile([128, 512], F32, tag="rmlp2", space="PSUM", bufs=2)
                nc.tensor.matmul(ps2[:, 0:w], lhsT=rW2(a), rhs=s1[:, 0:w],
                                 start=True, stop=True)
                nc.vector.tensor_copy(radT[:, o:o + w], ps2[:, 0:w])

        # ---------------- phase 0: iota, S, distT, xT init ----------------
        with tc.tile_pool(name="ph0", bufs=1) as ph0, \
             tc.tile_pool(name="ph0s", bufs=2) as ph0s:
            iota_i = ph0.tile([128, 128], I32, tag="iotai")
            nc.gpsimd.iota(iota_i[:], pattern=[[1, 128]], base=0, channel_multiplier=0)
            iota_f = ph0.tile([128, 128], F32, tag="iotaf")
            nc.vector.tensor_copy(iota_f[:], iota_i[:])
            embT = ph0.tile([128, NP], F32, tag="embT")
            nc.sync.dma_start(embT[:], embT_d[:])
            for g in range(2):
                nc.gpsimd.memset(xT[g][:], 0.0)
                nc.vector.tensor_copy(
                    xT[g][:].rearrange("p (n k) -> p n k", k=NC49)[:, :, 0:1],
                    embT[:, g * 128:(g + 1) * 128].rearrange("p n -> p n ()"))

            # S (edge->node one-hot) from dstS column
            for et in range(NT):
                g = et // TG
                dloc = ph0s.tile([128, 1], F32, tag="dloc")
                nc.vector.tensor_copy(dloc[:], idxs[:, 2 * NT + et:2 * NT + et + 1])
                nc.vector.tensor_scalar_add(dloc[:], dloc[:], float(-128 * g))
                nc.vector.tensor_tensor(S_all[:, et * 128:(et + 1) * 128],
                                        dloc[:].to_broadcast([128, 128]), iota_f[:],
                                        op=ALU.is_equal)

            # distT = exp(G*(d - off)^2), [120, 5*EP]
            dbc = ph0.tile([120, EP], F32, tag="dbc")
            nc.sync.dma_start(dbc[:], d_row_d[0:1, :].to_broadcast([120, EP]))
            distF = ph0.tile([120, 5 * EP], F32, tag="distF")
            for ci in range(5):
                nc.vector.tensor_scalar(distF[:, ci * EP:(ci + 1) * EP], dbc[:],
                                        offc[:, ci:ci + 1], None, op0=ALU.subtract)
            nc.scalar.activation(distF[:], distF[:], AF.Square)
            nc.scalar.activation(distT[:], distF[:], AF.Exp, scale=GCOEF)

        # ---------------- phase A: edge-degree embedding ----------------
        skip_deg = "edgedeg" in ablate
        with tc.tile_pool(name="phA", bufs=1) as phA, \
             tc.tile_pool(name="phAs", bufs=2) as phAs:
            Yt_all = phA.tile([128, NT * NC49], BF, tag="Yt_all")
            nc.sync.dma_start(Yt_all[:], Yt_d[:])
            s2_all = phA.tile([128, EP], BF, tag="s2a")
            radD = phA.tile([128, NT * 896], BF, tag="radD")
            with tc.tile_pool(name="phAp", bufs=1, space="PSUM") as phAp:
                for o, w in ([] if skip_deg else _chunks(EP, 512)):
                    ps = phAp.tile([128, 512], F32, tag="mlp1", space="PSUM", bufs=2)
                    for ci in range(5):
                        nc.tensor.matmul(ps[:, 0:w], lhsT=dW1(ci),
                                         rhs=distT[:, ci * EP + o:ci * EP + o + w],
                                         start=(ci == 0), stop=(ci == 4))
                    s1 = phAs.tile([128, 512], BF, tag="s1")
                    nc.scalar.activation(s1[:, 0:w], ps[:, 0:w], AF.Silu, bias=degb1)
                    ps2 = phAp.tile([128, 512], F32, tag="mlp2", space="PSUM", bufs=2)
                    nc.tensor.matmul(ps2[:, 0:w], lhsT=degw2, rhs=s1[:, 0:w],
                                     start=True, stop=True)
                    nc.scalar.activation(s2_all[:, o:o + w], ps2[:, 0:w], AF.Silu, bias=degb2)
                for et in ([] if skip_deg else range(NT)):
                    ps3 = phAp.tile([128, 896], F32, tag="mlp3", space="PSUM", bufs=2)
                    for o, s in _chunks(7 * C):
                        nc.tensor.matmul(ps3[:, o:o + s],
                                         lhsT=s2_all[:, et * 128:(et + 1) * 128],
                                         rhs=degw3[:, o:o + s], start=True, stop=True)
                    copy_eng(et, radD[:, et * 896:(et + 1) * 896], ps3[:])

            # scatter: PSUM-accumulated over tiles, per 8-coeff chunk
            for g in ([] if skip_deg else range(2)):
                with tc.tile_pool(name=f"degp{g}", bufs=2, space="PSUM") as degp, \
                     tc.tile_pool(name=f"degt{g}", bufs=2, space="PSUM") as degt, \
                     tc.tile_pool(name=f"degs{g}", bufs=2) as degs:
                    for k0 in range(0, NC49, 8):
                        nk = min(8, NC49 - k0)
                        acc = degp.tile([128, 1024], F32, tag="dacc", space="PSUM")
                        for ti in range(TG):
                            et = g * TG + ti
                            M = degs.tile([128, 1024], BF, tag="M")
                            for l in range(L_MAX + 1):
                                ks, kc = LBLK[l]
                                lo, hi = max(ks, k0), min(ks + kc, k0 + nk)
                                if lo >= hi:
                                    continue
                                tt_eng(ti + l).tensor_tensor(
                                    M[:, (lo - k0) * 128:(hi - k0) * 128]
                                        .rearrange("p (k c) -> p k c", c=128),
                                    Yt_all[:, et * NC49 + lo:et * NC49 + hi]
                                        .rearrange("p k -> p k ()").to_broadcast([128, hi - lo, 128]),
                                    radD[:, et * 896 + l * 128:et * 896 + (l + 1) * 128]
                                        .rearrange("p c -> p () c").to_broadcast([128, hi - lo, 128]),
                                    op=ALU.mult)
                            for o, s in _chunks(nk * 128):
                                nc.tensor.matmul(acc[:, o:o + s], lhsT=S_all[:, et * 128:(et + 1) * 128],
                                                 rhs=M[:, o:o + s], start=(ti == 0), stop=(ti == TG - 1))
                        dchunk = degs.tile([128, 1024], BF, tag="dchunk")
                        copy_eng(k0 // 8, dchunk[:, 0:nk * 128], acc[:, 0:nk * 128])
                        tp = degt.tile([128, 1024], BF, tag="dtp", space="PSUM")
                        for j in range(nk):
                            nc.tensor.transpose(tp[:, j * 128:(j + 1) * 128],
                                                dchunk[:, j * 128:(j + 1) * 128], ident_b[:])
                        xs = xT[g][:].rearrange("p (n k) -> p n k", k=NC49)[:, :, k0:k0 + nk]
                        nc.vector.tensor_tensor(
                            xs, xs, tp[:, 0:nk * 128].rearrange("p (j n) -> p n j", j=nk),
                            op=ALU.add)
        with tc.tile_pool(name="phY", bufs=1) as phy, \
             tc.tile_pool(name="phYp", bufs=1, space="PSUM") as phyp:
            sq_t = phy.tile([128, 3136], BF, tag="ph_sq")
            redA = [phy.tile([128, 896], BF, tag=f"ph_red{g}", name=f"ph_red{g}")
                    for g in range(2)]
            stats_pre(0, sq_t, redA[0])
            stats_pre(1, sq_t, redA[1])
            stats_fin("att0", 0, 0, redA[0], phyp, "ph_ms")
            ys_rows(0, 0)
            stats_fin("att0", 0, 1, redA[1], phyp, "ph_ms")
            ys_rows(0, 1)
        dbg("xT0", xT[0][:])
        dbg("xT1", xT[1][:])

        # ---------- attention ----------
        def attention(a, nidx, feed=None):
            last = (a == 2)
            esA = ExitStack()
            ap_ = esA.enter_context(tc.tile_pool(name=f"at{a}", bufs=1))
            log_all = ap_.tile([128, NT * 8], F32, tag="log_all")
            radT = ap_.tile([128, EP], BF, tag="radT")

            # --- radial MLP (norm/y-rows/AG already ran in the prior phase's hook) ---
            with tc.tile_pool(name=f"at{a}r", bufs=2) as rs, \
                 tc.tile_pool(name=f"at{a}rp", bufs=1, space="PSUM") as rp:
                rad_mlp(a, radT, rs, rp)

            # --- edge phase ---
            mp = esA.enter_context(tc.tile_pool(name=f"at{a}m", bufs=1))
            esP = ExitStack()
            pp = esP.enter_context(tc.tile_pool(name=f"at{a}p", bufs=1, space="PSUM"))
            agn = {}

            def alloc_agn(g):
                # last attention keeps both groups' agg; agn1 reuses the ms2
                # rotation (allocated after the final remote gather).
                if last and g == 1:
                    agn[g] = mp.tile([128, W29], BF, tag="ms2", bufs=2, name="agn1")
                elif last:
                    agn[g] = mp.tile([128, W29], BF, tag="agn0", bufs=1, name="agn0")
                else:
                    agn[g] = mp.tile([128, W29], BF, tag="agnX", bufs=1, name=f"agn{g}")

            def feed_pre(g):
                if feed is None:
                    return
                sq_t = mp.tile([128, W29], BF, tag="mtt", bufs=2, name="sq_t")
                red = per.tile([128, 896], BF, tag="redP", bufs=2, name=f"red_{feed}_{g}")
                stats_pre(g, sq_t, red)
                pend_red[(feed, g)] = red

            def group_gather(g):
                vs = [mp.tile([128, W29], BF, tag=f"vsb_{ti}", name=f"vsb{ti}", bufs=1)
                      for ti in range(TG)]
                for ti in range(TG):
                    et = g * TG + ti
                    nc.gpsimd.indirect_dma_start(
                        out=vs[ti][:], out_offset=None, in_=yt_loc[:],
                        in_offset=bass.IndirectOffsetOnAxis(ap=idxs[:, NT + et:NT + et + 1], axis=0))
                return vs

            def group_addrem(g, vs):
                for ti in range(TG):
                    et = g * TG + ti
                    m2 = mp.tile([128, W29], BF, tag="ms2", bufs=2)
                    nc.gpsimd.indirect_dma_start(
                        out=m2[:], out_offset=None, in_=ys_full[:],
                        in_offset=bass.IndirectOffsetOnAxis(ap=idxs[:, et:et + 1], axis=0))
                    nc.vector.tensor_tensor(vs[ti][:], vs[ti][:], m2[:], op=ALU.add)
                if a == 0 and g == 0:
                    dbg("gat0", vs[0][:])

            def tile_T(g, ti, vs):
                et = g * TG + ti
                mt = mp.tile([128, W29], BF, tag="mtt", bufs=2)
                for bi, (r0, nr) in enumerate(RCH8):
                    accT = pp.tile([128, 1024], BF, tag="accT", space="PSUM", bufs=4)
                    for j in range(nr):
                        nc.tensor.transpose(accT[:, j * 128:(j + 1) * 128],
                                            vs[ti][:, (r0 + j) * 128:(r0 + j + 1) * 128],
                                            ident_b[:])
                    nc.vector.tensor_tensor(
                        mt[:, r0 * 128:(r0 + nr) * 128].rearrange("p (r e) -> p r e", e=128),
                        accT[:, 0:nr * 128].rearrange("p (r e) -> p r e", e=128),
                        radT[:, et * 128:(et + 1) * 128].rearrange("p e -> p () e")
                            .to_broadcast([128, nr, 128]),
                        op=ALU.mult)
                if a == 0 and et == 0:
                    dbg("msg00", mt[:])
                return mt

            def tile_V(g, ti, vs, mt):
                et = g * TG + ti
                for bi, (r0, nr) in enumerate(RCH4):
                    accV = pp.tile([128, 512], F32, tag="accV", space="PSUM", bufs=2)
                    for j in range(nr):
                        nc.tensor.matmul(accV[:, j * 128:(j + 1) * 128],
                                         lhsT=mt[:, (r0 + j) * 128:(r0 + j + 1) * 128],
                                         rhs=wV(a), start=True, stop=True)
                    nc.scalar.copy(vs[ti][:, r0 * 128:(r0 + nr) * 128], accV[:, 0:nr * 128])
                qs = mp.tile([128, 128], BF, tag="qs", bufs=2)
                nc.scalar.activation(qs[:], mt[:, 0:128], AF.Silu)
                nc.vector.tensor_scalar(qs[:], qs[:], avecC[:, a:a + 1], None, op0=ALU.mult)
                sx = pp.tile([128, 512], F32, tag="sx", space="PSUM", bufs=2)
                nc.tensor.matmul(sx[:, 0:8], lhsT=qs[:], rhs=Hsel, start=True, stop=True)
                nc.scalar.copy(log_all[:, et * 8:(et + 1) * 8], sx[:, 0:8])
                if a == 0 and et == 0:
                    dbg("vsb00", vs[0][:])

            def group_TV(g, vs):
                mt_prev = tile_T(g, 0, vs)
                for ti in range(1, TG):
                    mt = tile_T(g, ti, vs)
                    tile_V(g, ti - 1, vs, mt_prev)
                    mt_prev = mt
                tile_V(g, TG - 1, vs, mt_prev)

            def softmax(g):
                # logits are bounded (|logit| <~ 24): exact softmax without the
                # max shift — alpha = exp(l)/sum exp(l) is shift-invariant.
                lsl = log_all[:, g * TG * 8:(g + 1) * TG * 8]
                exs = mp.tile([128, TG * 8], BF, tag="exs", bufs=2)
                nc.scalar.activation(exs[:], lsl, AF.Exp)
                return exs

            def sh8_build(g, exs):
                sh = [mp.tile([128, 1024], BF, tag=f"sh8_{ti}", name=f"sh8{ti}", bufs=1)
                      for ti in range(TG)]
                for ti in range(TG):
                    et = g * TG + ti
                    nc.gpsimd.tensor_tensor(
                        sh[ti][:].rearrange("p (h n) -> p h n", h=8),
                        S_all[:, et * 128:(et + 1) * 128].rearrange("p n -> p () n")
                            .to_broadcast([128, 8, 128]),
                        exs[:, ti * 8:(ti + 1) * 8].rearrange("p h -> p h ()")
                            .to_broadcast([128, 8, 128]),
                        op=ALU.mult)
                return sh

            def scatter(g, exs, vs, sh):
                alloc_agn(g)
                dps = pp.tile([128, 512], F32, tag="sx", space="PSUM", bufs=2)
                for ti in range(TG):
                    et = g * TG + ti
                    nc.tensor.matmul(dps[:, 0:8], lhsT=S_all[:, et * 128:(et + 1) * 128],
                                     rhs=exs[:, ti * 8:(ti + 1) * 8],
                                     start=(ti == 0), stop=(ti == TG - 1))
                rden = mp.tile([128, 8], F32, tag="rden", bufs=2)
                nc.vector.tensor_scalar_max(rden[:], dps[:, 0:8], 1e-9)
                nc.vector.reciprocal(rden[:], rden[:])
                agv = agn[g][:].rearrange("p (r h d) -> p h r d", h=8, d=16)
                for h2 in range(HEADS):
                    shacc = pp.tile([128, 512], F32, tag="sx", space="PSUM", bufs=2)
                    for ti in range(TG):
                        nc.tensor.matmul(
                            shacc[:, 0:NR * VPH],
                            lhsT=sh[ti][:, h2 * 128:(h2 + 1) * 128],
                            rhs=vs[ti][:].rearrange("p (r h d) -> p h r d", h=8, d=16)[:, h2],
                            start=(ti == 0), stop=(ti == TG - 1))
                    if h2 % 2 == 0:
                        nc.vector.tensor_scalar(agv[:, h2],
                                                shacc[:, 0:NR * VPH].rearrange("p (r d) -> p r d", d=16),
                                                rden[:, h2:h2 + 1], None, op0=ALU.mult)
                    else:
                        nc.scalar.activation(agv[:, h2],
                                             shacc[:, 0:NR * VPH].rearrange("p (r d) -> p r d", d=16),
                                             AF.Copy, scale=rden[:, h2:h2 + 1])
                if a == 0 and g == 0:
                    dbg("agg00", agn[0][:])

            def project(g):
                ag = mp.tile([128, W29], BF, tag="mtt", bufs=2)
                for bi, (r0, nr) in enumerate(RCH8):
                    acc = pp.tile([128, 1024], BF, tag="accT", space="PSUM", bufs=4)
                    for j in range(nr):
                        nc.tensor.transpose(acc[:, j * 128:(j + 1) * 128],
                                            agn[g][:, (r0 + j) * 128:(r0 + j + 1) * 128],
                                            ident_b[:])
                    copy_eng(bi, ag[:, r0 * 128:(r0 + nr) * 128], acc[:, 0:nr * 128])
                for ci, (o, s) in enumerate(_chunks(W29)):
                    wacc = pp.tile([128, 512], F32, tag="sx", space="PSUM", bufs=2)
                    nc.tensor.matmul(wacc[:, 0:s], lhsT=wP(a), rhs=ag[:, o:o + s],
                                     start=True, stop=True)
                    r0, r1 = o // 128, (o + s) // 128
                    for (os_, ks, cnt) in RBLK:
                        lo, hi = max(os_, r0), min(os_ + cnt, r1)
                        if lo >= hi:
                            continue
                        xv = xT[g][:].rearrange("p (n k) -> p n k", k=NC49)[
                            :, :, ks + (lo - os_):ks + (hi - os_)]
                        nc.vector.tensor_tensor(
                            xv, xv,
                            wacc[:, 0:s].rearrange("p (r n) -> p n r", n=128)[
                                :, :, lo - r0:hi - r0],
                            op=ALU.add)

            # ---- group pipeline ----
            vs0 = group_gather(0)
            group_addrem(0, vs0)
            group_TV(0, vs0)
            exs0 = softmax(0)
            sh0 = sh8_build(0, exs0)
            scatter(0, exs0, vs0, sh0)
            if not last:
                project(0)
            feed_pre(0)
            vs1 = group_gather(1)
            group_addrem(1, vs1)
            group_TV(1, vs1)
            exs1 = softmax(1)
            sh1 = sh8_build(1, exs1)
            scatter(1, exs1, vs1, sh1)
            if not last:
                project(1)
            feed_pre(1)
            if a == 0:
                dbg("logits0", log_all[:])

            if last:
                # pool-first epilogue: pooled[C, (r G)] = w_p^T @ (PT^T @ agn)^T
                esP.close()
                with tc.tile_pool(name="poolEp", bufs=1, space="PSUM") as pep:
                    p2 = pep.tile([16, W29], F32, tag="p2", space="PSUM")
                    for o, s in _chunks(W29):
                        for g in range(2):
                            nc.tensor.matmul(p2[:, o:o + s], lhsT=PT[g][:],
                                             rhs=agn[g][:, o:o + s],
                                             start=(g == 0), stop=(g == 1))
                    p2sb = mp.tile([16, W29], BF, tag="mtt", name="p2sb", bufs=2)
                    nc.vector.tensor_copy(p2sb[:], p2[:])
                with tc.tile_pool(name="poolFp", bufs=1, space="PSUM") as pfp:
                    ptp = pfp.tile([128, NR * G], BF, tag="ptp", space="PSUM")
                    for r in range(NR):
                        nc.tensor.transpose(ptp[:, r * G:(r + 1) * G],
                                            p2sb[:, r * 128:(r + 1) * 128],
                                            ident_b[0:16, 0:16])
                    p2T = mp.tile([128, NR * G], BF, tag="sh8_0", name="p2T", bufs=1)
                    nc.scalar.copy(p2T[:], ptp[:])
                    pps = pfp.tile([128, 512], F32, tag="pps", space="PSUM")
                    nc.tensor.matmul(pps[:, 0:NR * G], lhsT=wP(2), rhs=p2T[:],
                                     start=True, stop=True)
                    pooled_sb = mp.tile([128, NR * G], F32, tag="sh8_1", name="pooled_sb", bufs=1)
                    nc.scalar.copy(pooled_sb[:], pps[:, 0:NR * G])
                    nc.sync.dma_start(pooled_d[:], pooled_sb[:])
                esA.close()
            else:
                esP.close()
                esA.close()

        # ---------- ffn ----------
        def ffn(i, nidx, nxt_a, nxt_nidx):
            key = f"ffn{i}"
            QW = 32 * NC49      # 1568 cols per quarter
            with tc.tile_pool(name=f"ff{i}", bufs=1) as fp:
                hfull = [fp.tile([128, W49], BF, tag=f"hf{g}", name=f"hf{g}") for g in range(2)]
                with tc.tile_pool(name=f"ff{i}fp", bufs=1, space="PSUM") as pfin:
                    for g in range(2):
                        stats_fin(key, nidx, g, pend_red.pop((key, g)), pfin, "ffms")
                for g in range(2):
                    rms_apply(False, hfull[g], g, pend_scl.pop((key, g)))
                sqN = fp.tile([128, 3136], BF, tag="ff_sq")
                redN = [fp.tile([128, 896], BF, tag=f"ff_red{g}", name=f"ff_red{g}")
                        for g in range(2)]
                for g in range(2):
                    with tc.tile_pool(name=f"ff{i}p{g}", bufs=1, space="PSUM") as ffp, \
                         tc.tile_pool(name=f"ff{i}s{g}", bufs=2) as fs:
                        for q in range(4):
                            hkn = hfull[g][:, q * QW:(q + 1) * QW]
                            sgs = [fs.tile([128, 32], BF, tag=f"sg{fc}", name=f"sg{fc}",
                                           bufs=1) for fc in range(4)]
                            # half-quarters split on k: (0,24) and (24,49)
                            for klo, khi in ((0, 24), (24, 49)):
                                hw_ = (khi - klo) * 32
                                ops = ffp.tile([128, 800], F32, tag="ops", space="PSUM")
                                h1gs = []

                                def mm1(fc):
                                    h1p = ffp.tile([128, 800], F32, tag="h1p",
                                                   space="PSUM", bufs=2)
                                    for o, s in _chunks(hw_):
                                        nc.tensor.matmul(h1p[:, o:o + s], lhsT=fW1(i, fc),
                                                         rhs=hkn[:, klo * 32 + o:klo * 32 + o + s],
                                                         start=True, stop=True)
                                    return h1p

                                def gate(fc, h1p):
                                    if klo == 0:
                                        nc.scalar.activation(sgs[fc][:], h1p[:, 0:32], AF.Sigmoid)
                                    h1g = fs.tile([128, 800], BF, tag="h1g")
                                    h1gs.append(h1g)
                                    if fc % 2 == 0:
                                        nc.vector.tensor_tensor(
                                            h1g[:, 0:hw_].rearrange("p (k n) -> p k n", n=32),
                                            h1p[:, 0:hw_].rearrange("p (k n) -> p k n", n=32),
                                            sgs[fc][:].rearrange("p n -> p () n")
                                                .to_broadcast([128, khi - klo, 32]),
                                            op=ALU.mult)
                                    else:
                                        h1c = fs.tile([128, 800], BF, tag="h1c")
                                        nc.scalar.copy(h1c[:, 0:hw_], h1p[:, 0:hw_])
                                        nc.gpsimd.tensor_tensor(
                                            h1g[:, 0:hw_].rearrange("p (k n) -> p k n", n=32),
                                            h1c[:, 0:hw_].rearrange("p (k n) -> p k n", n=32),
                                            sgs[fc][:].rearrange("p n -> p () n")
                                                .to_broadcast([128, khi - klo, 32]),
                                            op=ALU.mult)

                                def mm2(fc):
                                    for o, s in _chunks(hw_):
                                        nc.tensor.matmul(ops[:, o:o + s], lhsT=fW2(i, fc),
                                                         rhs=h1gs[fc][:, o:o + s],
                                                         start=(fc == 0), stop=(fc == 3))

                                h1p_prev = mm1(0)
                                gate(0, h1p_prev)
                                h1p_prev = mm1(1)
                                mm2(0)
                                gate(1, h1p_prev)
                                h1p_prev = mm1(2)
                                mm2(1)
                                gate(2, h1p_prev)
                                h1p_prev = mm1(3)
                                mm2(2)
                                gate(3, h1p_prev)
                                mm2(3)
                                xsl = xT[g][:, q * QW:(q + 1) * QW] \
                                    .rearrange("p (n k) -> p n k", k=NC49)[:, :, klo:khi]
                                if klo == 0:
                                    nc.vector.tensor_tensor(
                                        xsl, xsl,
                                        ops[:, 0:hw_].rearrange("p (k n) -> p n k", n=32),
                                        op=ALU.add)
                                else:
                                    oc = fs.tile([128, 800], F32, tag="oc")
                                    nc.scalar.copy(oc[:, 0:hw_], ops[:, 0:hw_])
                                    nc.gpsimd.tensor_tensor(
                                        xsl, xsl,
                                        oc[:, 0:hw_].rearrange("p (k n) -> p n k", n=32),
                                        op=ALU.add)
                    stats_pre(g, sqN, redN[g])
                with tc.tile_pool(name=f"ff{i}yp", bufs=1, space="PSUM") as pfy:
                    for g in range(2):
                        stats_fin(f"att{nxt_a}", nxt_nidx, g, redN[g], pfy, "ffms")
                        ys_rows(nxt_a, g)

        attention(0, 0, feed="ffn0")
        dbg("xT0_a0", xT[0][:])
        ffn(0, 1, 1, 2)
        dbg("xT0_f0", xT[0][:])
        attention(1, 2, feed="ffn1")
        ffn(1, 3, 2, 4)
        dbg("xT0_l1", xT[0][:])
        dbg("xT1_l1", xT[1][:])
        attention(2, 4)

    nc.compile()
    return nc


_CACHE = {}


def _get_program(meta, debug=()):
    key = (meta["TG"], tuple(n for n, _ in debug), tuple(sorted(ABLATE)))
    if key not in _CACHE:
        _CACHE[key] = build_program(meta, debug, frozenset(ABLATE))
    return _CACHE[key]


DEBUG_OUTS = ()
ABLATE = set()


class _Runner:
    """Caches the jitted shard_map callable for a compiled program."""

    def __init__(self, nc):
        import jax
        from jax.sharding import Mesh, PartitionSpec
        from jax.experimental.shard_map import shard_map
        from concourse.bass2jax import _bass_exec_p, install_neuronx_cc_hook, partition_id_tensor
        install_neuronx_cc_hook()
        self.jax = jax
        pname = nc.partition_id_tensor.name if nc.partition_id_tensor else None
        in_names, out_names, out_avals, zeros = [], [], [], []
        for alloc in nc.m.functions[0].allocations:
            if not isinstance(alloc, mybir.MemoryLocationSet):
                continue
            name = alloc.memorylocations[0].name
            if alloc.kind == "ExternalInput":
                if name != pname:
                    in_names.append(name)
            elif alloc.kind == "ExternalOutput":
                out_names.append(name)
                shp = tuple(alloc.tensor_shape)
                dt = mybir.dt.np(alloc.dtype)
                out_avals.append(jax.core.ShapedArray(shp, dt))
                zeros.append(np.zeros((NCORES * shp[0],) + shp[1:], dt))
        self.in_names, self.out_names, self.zeros = in_names, out_names, zeros
        n_params, n_outs = len(in_names), len(out_names)
        names_all = in_names + out_names + ([pname] if pname else [])

        def _body(*args):
            operands = list(args)
            if pname is not None:
                operands.append(partition_id_tensor())
            return tuple(_bass_exec_p.bind(
                *operands, out_avals=tuple(out_avals), in_names=tuple(names_all),
                out_names=tuple(out_names), lowering_input_output_aliases=(),
                sim_require_finite=True, sim_require_nnan=True, nc=nc))

        devices = jax.devices()[:NCORES]
        self.mesh = Mesh(np.asarray(devices), ("core",))
        self.fn = jax.jit(shard_map(
            _body, mesh=self.mesh,
            in_specs=(PartitionSpec("core"),) * (n_params + n_outs),
            out_specs=(PartitionSpec("core"),) * n_outs, check_rep=False),
            keep_unused=True)

    def stage(self, in_maps):
        from jax.sharding import NamedSharding, PartitionSpec
        sh = NamedSharding(self.mesh, PartitionSpec("core"))
        args = [np.concatenate([np.asarray(m[n]) for m in in_maps], axis=0)
                for n in self.in_names] + list(self.zeros)
        return [self.jax.device_put(a, sh) for a in args]

    def __call__(self, staged):
        return self.fn(*staged)

    def results(self, outs):
        res = [dict() for _ in range(NCORES)]
        for i, n in enumerate(self.out_names):
            arr = np.asarray(outs[i])
            per = arr.reshape(NCORES, arr.shape[0] // NCORES, *arr.shape[1:])
            for c in range(NCORES):
                res[c][n] = per[c]
        return res


_RUNNERS = {}


def get_runner(meta, debug=()):
    key = (meta["TG"], tuple(n for n, _ in debug))
    if key not in _RUNNERS:
        _RUNNERS[key] = _Runner(_get_program(meta, debug))
    return _RUNNERS[key]


def kernel(**inputs):
    meta, in_maps = host_prep(inputs)
    runner = get_runner(meta, DEBUG_OUTS)
    staged = runner.stage(in_maps)
    runner(staged)          # warmup dispatch
    outs = runner(staged)
    self_results = runner.results(outs)
    # pooled is [C, (r G)] per core; sum cores, then out[g, R[r], c]
    acc = np.zeros((128, NR * G), np.float64)
    for c in range(NCORES):
        acc += self_results[c]["pooled"].astype(np.float64)
    acc = acc.reshape(128, NR, G)            # [c, r, g]
    out = np.zeros((G, NC49, C), np.float32)
    out[:, RESTRICT_NP, :] = acc.transpose(2, 1, 0).astype(np.float32)
    kernel.last_results = self_results
    kernel.last_runner = runner
    kernel.last_staged = staged
    return out.reshape(1, -1)


# revision 49
# speedup vs baseline: 1.0570x; 1.0570x over previous
"""EquiformerV2 (2-layer) Bass/Tile kernel for 8 trn2 NeuronCores — v3.

Sharding: dst-node-range parallel (core c owns nodes [256c, 256c+256) and the
edges terminating there). Per attention: y_s/y_t computed locally in bf16, one
AllGather of y_s, then per 128-edge tile gathered messages feed transposes,
values, logits and a one-hot-matmul scatter.

v3 vs v2: packed weight loads (2 DMAs), radial MLPs batched to 512-col
streams with bias folded into the activation, rad modulation fused into the
transpose-PSUM drain (h-major rad), software-pipelined edge loop (T(i+1)
issued before V(i)), engine-partitioned drains (vector=T, scalar=V), FFN gate
reads PSUM directly, Rsqrt-based norms, pool-first latent epilogue.
"""
import math
from contextlib import ExitStack

import numpy as np

import concourse.bass as bass
import concourse.bacc as bacc
import concourse.mybir as mybir
import concourse.tile as tile
from concourse.bass_utils import run_bass_kernel_spmd
from concourse.masks import make_identity

F32 = mybir.dt.float32
BF = mybir.dt.bfloat16
I32 = mybir.dt.int32
AF = mybir.ActivationFunctionType
ALU = mybir.AluOpType
AX = mybir.AxisListType
BF_NP = mybir.dt.np(BF)

NCORES = 8
L_MAX, M_MAX = 6, 2
NC49 = (L_MAX + 1) ** 2
C = 128
H = 128
HEADS, VPH = 8, 16
FFN = 512
NB = 600
N, E, G = 2048, 12288, 16
NP = N // NCORES
AVG_DEG = 3.0
CUTOFF = 5.0
DISC_LO, DISC_HI = -3.26267, 3.295396
EPS = 1e-6

LBLK = [(l * l, 2 * l + 1) for l in range(L_MAX + 1)]
RBLK = []
_r = 0
for _l in range(L_MAX + 1):
    _cnt = min(2 * _l + 1, 2 * M_MAX + 1)
    RBLK.append((_r, _l * _l + _l - min(_l, M_MAX), _cnt))
    _r += _cnt
NR = _r                   # 29
W29 = NR * 128
W49 = NC49 * 128
RCH8 = [(r0, min(8, NR - r0)) for r0 in range(0, NR, 8)]
RCH4 = [(r0, min(4, NR - r0)) for r0 in range(0, NR, 4)]

_off_np = np.linspace(0.0, CUTOFF, NB).astype(np.float32)
GCOEF = float(-0.5 / (2.0 * (_off_np[1] - _off_np[0])) ** 2)
_mv_np = np.array([m for l in range(L_MAX + 1) for m in range(-l, l + 1)])
_deg_np = np.array([l for l in range(L_MAX + 1) for m in range(-l, l + 1)])
RESTRICT_NP = np.nonzero(np.abs(_mv_np) <= M_MAX)[0]

# bf16 weight pack layout: (name, cols). [120-row blocks live in rows 0:120.]
PACKB = [
    ("w_st", 3 * 256), ("w_v", 3 * 128), ("w_p", 3 * 128),
    ("rad_w2", 3 * 128), ("Hsel", 8), ("ffn_w1", 2 * FFN), ("ffn_w2", 8 * 128),
    ("degw2", 128), ("degw3", 7 * C), ("degw1c", 5 * 128), ("radw1c", 15 * 128),
]
PBOFF = {}
_o = 0
for _n, _w in PACKB:
    PBOFF[_n] = _o
    _o += _w
PBW = _o
# f32 pack: nwT 35 | avecC 3 | offc 5 (rows 0:120) | degb1 1 | degb2 1 | radb1 3
PFW = 48


def real_sph_harm_np(vec):
    r = np.linalg.norm(vec, axis=-1, keepdims=True)
    u = vec / np.maximum(r, 1e-8)
    x, y, z = u[:, 0], u[:, 1], u[:, 2]
    ct = np.clip(z, -1.0, 1.0)
    st = np.sqrt(np.clip(1.0 - ct * ct, 1e-12, 1.0))
    phi = np.arctan2(y, x)
    P = {(0, 0): np.ones_like(ct)}
    for m in range(1, L_MAX + 1):
        P[(m, m)] = -(2 * m - 1) * st * P[(m - 1, m - 1)]
    for m in range(0, L_MAX):
        P[(m + 1, m)] = (2 * m + 1) * ct * P[(m, m)]
    for m in range(0, L_MAX + 1):
        for l in range(m + 2, L_MAX + 1):
            P[(l, m)] = ((2 * l - 1) * ct * P[(l - 1, m)] - (l + m - 1) * P[(l - 2, m)]) / (l - m)
    cols = []
    for l in range(L_MAX + 1):
        for m in range(-l, l + 1):
            am = abs(m)
            nrm = math.sqrt((2 * l + 1) / (4 * math.pi) * math.factorial(l - am) / math.factorial(l + am))
            if m == 0:
                cols.append(nrm * P[(l, 0)])
            elif m > 0:
                cols.append(math.sqrt(2.0) * nrm * P[(l, m)] * np.cos(m * phi))
            else:
                cols.append(math.sqrt(2.0) * nrm * P[(l, am)] * np.sin(am * phi))
    return np.stack(cols, axis=-1).astype(np.float32)


def host_prep(inputs):
    f = lambda k: np.asarray(inputs[k], np.float32)
    pos = f("pos")
    edge_vec = f("edge_vec")
    edge_index = np.asarray(inputs["edge_index"]).astype(np.int64)
    batch = np.asarray(inputs["batch"]).astype(np.int64)

    src, dst = edge_index[0], edge_index[1]
    d_all = np.linalg.norm(edge_vec, axis=-1).astype(np.float32)
    Y_all = (real_sph_harm_np(edge_vec) / np.float32(AVG_DEG)).astype(np.float32)

    t = np.clip(np.round((pos - DISC_LO) / (DISC_HI - DISC_LO) * 128.0 - 0.5), 0, 127).astype(np.int64)
    et_ = f("embed_table")
    emb = (et_[t[:, 0]] + et_[t[:, 1]] + et_[t[:, 2]]).astype(np.float32)

    core_of = dst // NP
    grp_of = (dst % NP) // 128
    lists = [[np.nonzero((core_of == c) & (grp_of == g))[0] for g in range(2)] for c in range(NCORES)]
    TG = max(1, (max(len(lists[c][g]) for c in range(NCORES) for g in range(2)) + 127) // 128)
    NT = 2 * TG
    EP = NT * 128

    cnt = np.bincount(batch, minlength=G).astype(np.float32)
    inv_cnt = (1.0 / np.maximum(cnt, 1.0)).astype(np.float32)

    # ---- f32 pack ----
    nws = [f("attn_norm_w")[0], f("ffn_norm_w")[0], f("attn_norm_w")[1], f("ffn_norm_w")[1], f("final_norm_w")]
    packF = np.zeros((128, PFW), np.float32)
    packF[:, 0:35] = np.concatenate([w.T for w in nws], axis=1)
    packF[:, 35:38] = np.stack([f("alpha_vec")[0].reshape(-1), f("alpha_vec")[1].reshape(-1),
                                f("lat_alpha").reshape(-1)], axis=1)
    packF[0:120, 38:43] = np.ascontiguousarray(_off_np.reshape(5, 120).T)
    packF[:, 43] = f("deg_b1")
    packF[:, 44] = f("deg_b2")
    packF[:, 45] = f("rad_b1")[0]
    packF[:, 46] = f("rad_b1")[1]
    packF[:, 47] = f("lat_rad_b1")

    # ---- bf16 pack ----
    def stack_lat(key, lat_key):
        return np.concatenate([f(key)[0], f(key)[1], f(lat_key)], axis=1)  # [128, 3*128]

    packB = np.zeros((128, PBW), np.float32)

    def put(name, arr, rows=128):
        o = PBOFF[name]
        packB[0:rows, o:o + arr.shape[1]] = arr

    ws_ = stack_lat("w_src", "lat_w_src")     # [128, 3*128]
    wt_ = stack_lat("w_tgt", "lat_w_tgt")
    wst = np.zeros((128, 3 * 256), np.float32)
    for a_ in range(3):
        wst[:, a_ * 256:a_ * 256 + 128] = ws_[:, a_ * 128:(a_ + 1) * 128]
        wst[:, a_ * 256 + 128:a_ * 256 + 256] = wt_[:, a_ * 128:(a_ + 1) * 128]
    put("w_st", wst)
    put("w_v", stack_lat("w_val", "lat_w_val"))
    put("w_p", stack_lat("w_proj", "lat_w_proj"))
    put("rad_w2", stack_lat("rad_w2", "lat_rad_w2"))
    Hsel = np.zeros((128, HEADS), np.float32)
    Hsel[np.arange(128), np.arange(128) // VPH] = 1.0
    put("Hsel", Hsel)
    put("ffn_w1", np.concatenate([f("ffn_w1")[0], f("ffn_w1")[1]], axis=1))
    w2 = np.concatenate([f("ffn_w2")[0], f("ffn_w2")[1]], axis=0)  # [1024, 128]
    put("ffn_w2", w2.reshape(8, 128, 128).transpose(1, 0, 2).reshape(128, 8 * 128))
    put("degw2", f("deg_w2"))
    put("degw3", f("deg_w3"))
    put("degw1c", f("deg_w1").reshape(5, 120, C).transpose(1, 0, 2).reshape(120, 5 * C), rows=120)
    rw1 = np.stack([f("rad_w1")[0], f("rad_w1")[1], f("lat_rad_w1")], axis=0)  # [3, 600, H]
    put("radw1c", rw1.reshape(3, 5, 120, H).transpose(2, 0, 1, 3).reshape(120, 15 * H), rows=120)

    shared = {"packF": packF, "packB": packB.astype(BF_NP)}

    in_maps = []
    for c in range(NCORES):
        srcg = np.zeros((EP,), np.int64)
        dstg = np.zeros((EP,), np.int64)
        dstS = np.full((EP,), 30000, np.int64)
        d_row = np.zeros((1, EP), np.float32)
        Yc = np.zeros((EP, NC49), np.float32)
        for g in range(2):
            idx = lists[c][g]
            o = g * TG * 128
            n = len(idx)
            srcg[o:o + n] = src[idx]
            dstg[o:o + n] = dst[idx] - c * NP
            dstS[o:o + n] = dst[idx] - c * NP
            d_row[0, o:o + n] = d_all[idx]
            Yc[o:o + n] = Y_all[idx]
        embT = np.ascontiguousarray(emb[c * NP:(c + 1) * NP].T)
        PT = np.zeros((NP, G), np.float32)
        nloc = np.arange(c * NP, (c + 1) * NP)
        PT[np.arange(NP), batch[nloc]] = inv_cnt[batch[nloc]]
        YtT = np.ascontiguousarray(Yc.reshape(NT, 128, NC49).transpose(1, 0, 2).reshape(128, NT * NC49))
        idxs = np.zeros((128, 3 * NT), np.int32)
        idxs[:, 0:NT] = srcg.reshape(NT, 128).T
        idxs[:, NT:2 * NT] = dstg.reshape(NT, 128).T
        idxs[:, 2 * NT:3 * NT] = dstS.reshape(NT, 128).T
        m = dict(shared)
        m.update({"embT": embT, "d_row": d_row, "Yt": YtT.astype(BF_NP),
                  "idxs": idxs, "PT": PT.astype(BF_NP)})
        in_maps.append(m)
    return {"TG": TG, "NT": NT, "EP": EP}, in_maps


def _chunks(total, step=512):
    o = 0
    while o < total:
        yield o, min(step, total - o)
        o += step


def build_program(meta, debug=(), ablate=frozenset()):
    TG, NT, EP = meta["TG"], meta["NT"], meta["EP"]
    nc = bacc.Bacc("TRN2", target_bir_lowering=False, debug=False, num_devices=NCORES)

    def din(name, shape, dt=F32):
        return nc.dram_tensor(name, shape, dt, kind="ExternalInput")

    packF_d = din("packF", [128, PFW])
    packB_d = din("packB", [128, PBW], BF)
    embT_d = din("embT", [128, NP])
    d_row_d = din("d_row", [1, EP])
    Yt_d = din("Yt", [128, NT * NC49], BF)
    idxs_d = din("idxs", [128, 3 * NT], I32)
    PT_d = din("PT", [NP, G], BF)

    pooled_d = nc.dram_tensor("pooled", [128, NR * G], F32, kind="ExternalOutput")
    dbg_d = {name: nc.dram_tensor("dbg_" + name, list(shape), F32, kind="ExternalOutput")
             for name, shape in debug}

    ys_loc = nc.dram_tensor("ys_loc", [NP, W29], BF)
    yt_loc = nc.dram_tensor("yt_loc", [NP, W29], BF)
    ys_full = nc.dram_tensor("ys_full", [N, W29], BF, addr_space="Shared")
    RG = [list(range(NCORES))]

    with tile.TileContext(nc) as tc, ExitStack() as es:
        per = es.enter_context(tc.tile_pool(name="persist", bufs=1))

        def dbg(name, ap):
            if name in dbg_d:
                if ap.dtype != F32:
                    nc.gpsimd.dma_start(dbg_d[name][:], ap)
                else:
                    nc.sync.dma_start(dbg_d[name][:], ap)

        # ---- persistent tiles ----
        PB = per.tile([128, PBW], BF, tag="PB")
        nc.sync.dma_start(PB[:], packB_d[:])
        PF = per.tile([128, PFW], F32, tag="PF")
        nc.scalar.dma_start(PF[:], packF_d[:])
        idxs = per.tile([128, 3 * NT], I32, tag="idxs")
        nc.gpsimd.dma_start(idxs[:], idxs_d[:])
        PT = [per.tile([128, G], BF, tag=f"PT{g}", name=f"PT{g}") for g in range(2)]
        for g in range(2):
            nc.scalar.dma_start(PT[g][:], PT_d[g * 128:(g + 1) * 128, :])

        def wST(a):
            return PB[:, PBOFF["w_st"] + a * 256:PBOFF["w_st"] + (a + 1) * 256]

        def wV(a):
            return PB[:, PBOFF["w_v"] + a * 128:PBOFF["w_v"] + (a + 1) * 128]

        def wP(a):
            return PB[:, PBOFF["w_p"] + a * 128:PBOFF["w_p"] + (a + 1) * 128]

        def rW2(a):
            return PB[:, PBOFF["rad_w2"] + a * 128:PBOFF["rad_w2"] + (a + 1) * 128]

        def rW1(a, ci):
            o = PBOFF["radw1c"] + (a * 5 + ci) * 128
            return PB[0:120, o:o + 128]

        def fW1(i, fc):
            o = PBOFF["ffn_w1"] + i * FFN + fc * 128
            return PB[:, o:o + 128]

        def fW2(i, fc):
            o = PBOFF["ffn_w2"] + (i * 4 + fc) * 128
            return PB[:, o:o + 128]

        Hsel = PB[:, PBOFF["Hsel"]:PBOFF["Hsel"] + HEADS]
        degw2 = PB[:, PBOFF["degw2"]:PBOFF["degw2"] + 128]
        degw3 = PB[:, PBOFF["degw3"]:PBOFF["degw3"] + 7 * C]

        def dW1(ci):
            o = PBOFF["degw1c"] + ci * 128
            return PB[0:120, o:o + 128]

        nwT = PF[:, 0:35]
        avecC = PF[:, 35:38]
        offc = PF[0:120, 38:43]
        degb1 = PF[:, 43:44]
        degb2 = PF[:, 44:45]

        def radb1(a):
            return PF[:, 45 + a:46 + a]

        ident = per.tile([128, 128], F32, tag="ident")
        make_identity(nc, ident[:])
        ident_b = per.tile([128, 128], BF, tag="identb")
        nc.vector.tensor_copy(ident_b[:], ident[:])
        ones1f = per.tile([1, 128], F32, tag="ones1f")
        nc.vector.memset(ones1f[:], 1.0)
        ones128b = per.tile([128, 128], BF, tag="ones128")
        nc.vector.memset(ones128b[:], 1.0)
        epsc = per.tile([128, 1], F32, tag="epsc")
        nc.vector.memset(epsc[:], EPS)

        xT = [per.tile([128, W49], F32, tag=f"xT{g}", name=f"xT{g}") for g in range(2)]
        S_all = per.tile([128, NT * 128], BF, tag="S_all")
        distT = per.tile([120, 5 * EP], BF, tag="distT")

        copy_rr = [nc.scalar, nc.vector]

        def copy_eng(i, out_ap, in_ap):
            e = copy_rr[i % len(copy_rr)]
            if e is nc.scalar:
                e.copy(out_ap, in_ap)
            else:
                e.tensor_copy(out_ap, in_ap)

        def tt_eng(i):
            return [nc.vector, nc.gpsimd][i % 2]

        # ---------- rms norm split: stats-pre (sq/red), stats-fin (ms/rsqrt) ----
        def stats_pre(g, sq_t, red):
            """red[c,(l n)] = bf16 partials of sum_k x^2; sq_t >=3136-col scratch."""
            QN = 1568
            with nc.allow_low_precision(reason="bf16 ms-reduce, 0.4% on rms"):
                for qi in range(4):
                    sqq = sq_t[:, (qi % 2) * QN:(qi % 2 + 1) * QN]
                    xq = xT[g][:, qi * QN:(qi + 1) * QN]
                    [nc.vector, nc.gpsimd][qi % 2].tensor_tensor(sqq, xq, xq, op=ALU.mult)
                    for l in range(L_MAX + 1):
                        ks, kc = LBLK[l]
                        nc.vector.tensor_reduce(
                            red[:, l * 128 + qi * 32:l * 128 + (qi + 1) * 32],
                            sqq.rearrange("p (n k) -> p n k", k=NC49)[:, :, ks:ks + kc],
                            axis=AX.X, op=ALU.add)

        def stats_fin(key, nidx, g, red, psp, psum_tag):
            scl = per.tile([128, 896], F32, tag="scl", bufs=2, name=f"scl_{key}_{g}")
            msA = psp.tile([128, 512], F32, tag=psum_tag, space="PSUM", bufs=2)
            nc.tensor.matmul(msA[:, 0:512], lhsT=ones128b[:], rhs=red[:, 0:512],
                             start=True, stop=True)
            msB = psp.tile([128, 512], F32, tag=psum_tag, space="PSUM", bufs=2)
            nc.tensor.matmul(msB[:, 0:384], lhsT=ones128b[:], rhs=red[:, 512:896],
                             start=True, stop=True)
            for l in range(L_MAX + 1):
                msl = msA[:, l * 128:(l + 1) * 128] if l < 4 else msB[:, (l - 4) * 128:(l - 3) * 128]
                nc.scalar.activation(scl[:, l * 128:(l + 1) * 128], msl, AF.Ln,
                                     bias=epsc[:], scale=float(1.0 / ((2 * l + 1) * C)))
            nc.scalar.activation(scl[:], scl[:], AF.Exp, scale=-0.5)
            for l in range(L_MAX + 1):
                sl = scl[:, l * 128:(l + 1) * 128]
                nc.vector.tensor_scalar(sl, sl, nwT[:, nidx * 7 + l:nidx * 7 + l + 1],
                                        None, op0=ALU.mult)
            pend_scl[(key, g)] = scl

        def rms_apply(restricted, out_tile, g, scl):
            stride = NR if restricted else NC49
            blocks = RBLK if restricted else [(ks, ks, kc) for (ks, kc) in LBLK]
            for l, (os_, ks, cnt) in enumerate(blocks):
                ov = out_tile[:].rearrange("p (n k) -> p n k", k=stride)[:, :, os_:os_ + cnt]
                xv = xT[g][:].rearrange("p (n k) -> p n k", k=NC49)[:, :, ks:ks + cnt]
                iv = scl[:, l * 128:(l + 1) * 128].rearrange("p n -> p n ()") \
                    .to_broadcast([128, 128, cnt])
                tt_eng(l).tensor_tensor(ov, xv, iv, op=ALU.mult)

        pend_scl = {}
        pend_red = {}

        # ---------- combined ys||yt rows for one group: 29 mm + drains + DMAs ----
        def yrows_st(hr, a, g, ypp, yss, dbg_pref=None):
            hv = hr[:].rearrange("p (n k) -> p k n", k=NR)
            ysrow = yss.tile([128, W29], BF, tag="ysrow", bufs=1)
            ytrow = yss.tile([128, W29], BF, tag="ytrow", bufs=1)
            for bi, (r0, nr) in enumerate(RCH4):
                yp = ypp.tile([128, 1024], F32, tag="yp", space="PSUM", bufs=2)
                for j in range(nr):
                    nc.tensor.matmul(yp[:, j * 256:(j + 1) * 256],
                                     lhsT=hv[:, r0 + j, :], rhs=wST(a),
                                     start=True, stop=True)
                copy_eng(bi, ysrow[:, r0 * 128:(r0 + nr) * 128]
                             .rearrange("p (j c) -> p j c", c=128),
                         yp[:].rearrange("p (j c) -> p j c", c=256)[:, 0:nr, 0:128])
                copy_eng(bi + 1, ytrow[:, r0 * 128:(r0 + nr) * 128]
                             .rearrange("p (j c) -> p j c", c=128),
                         yp[:].rearrange("p (j c) -> p j c", c=256)[:, 0:nr, 128:256])
            if dbg_pref:
                dbg("ysr0", ysrow[:])
                dbg("ytr0", ytrow[:])
            nc.sync.dma_start(ys_loc[g * 128:(g + 1) * 128, :], ysrow[:])
            if g == 1 and "ag" not in ablate:
                nc.gpsimd.collective_compute(
                    "AllGather", ALU.bypass, replica_groups=RG,
                    ins=[ys_loc[:]], outs=[ys_full[:]])
            nc.scalar.dma_start(yt_loc[g * 128:(g + 1) * 128, :], ytrow[:])

        def ys_rows(a, g):
            """apply norm + ys/yt rows + AG part for group g (stats precomputed)."""
            scl = pend_scl.pop(("att%d" % a, g))
            with tc.tile_pool(name=f"ys{a}g{g}", bufs=1) as sbp:
                hrT = sbp.tile([128, W29], BF, tag="hrT")
                rms_apply(True, hrT, g, scl)
                with tc.tile_pool(name=f"ys{a}g{g}p", bufs=1, space="PSUM") as ypp:
                    yrows_st(hrT, a, g, ypp, sbp,
                             dbg_pref=(a == 0 and g == 0))

        # ---------- radial MLP for attention a (h-major output) ----------
        def rad_mlp(a, radT, rs, rp):
            for o, w in _chunks(EP, 512):
                ps = rp.tile([128, 512], F32, tag="rmlp1", space="PSUM", bufs=2)
                for ci in range(5):
                    nc.tensor.matmul(ps[:, 0:w], lhsT=rW1(a, ci),
                                     rhs=distT[:, ci * EP + o:ci * EP + o + w],
                                     start=(ci == 0), stop=(ci == 4))
                s1 = rs.tile([128, 512], BF, tag="rm_s1")
                nc.scalar.activation(s1[:, 0:w], ps[:, 0:w], AF.Silu, bias=radb1(a))
                ps2 = rp.tile([128, 512], F32, tag="rmlp2", space="PSUM", bufs=2)
                nc.tensor.matmul(ps2[:, 0:w], lhsT=rW2(a), rhs=s1[:, 0:w],
                                 start=True, stop=True)
                nc.vector.tensor_copy(radT[:, o:o + w], ps2[:, 0:w])

        # ---------------- phase 0: iota, S, distT, xT init ----------------
        with tc.tile_pool(name="ph0", bufs=1) as ph0, \
             tc.tile_pool(name="ph0s", bufs=2) as ph0s:
            iota_i = ph0.tile([128, 128], I32, tag="iotai")
            nc.gpsimd.iota(iota_i[:], pattern=[[1, 128]], base=0, channel_multiplier=0)
            iota_f = ph0.tile([128, 128], F32, tag="iotaf")
            nc.vector.tensor_copy(iota_f[:], iota_i[:])
            embT = ph0.tile([128, NP], F32, tag="embT")
            nc.sync.dma_start(embT[:], embT_d[:])
            for g in range(2):
                nc.gpsimd.memset(xT[g][:], 0.0)
                nc.vector.tensor_copy(
                    xT[g][:].rearrange("p (n k) -> p n k", k=NC49)[:, :, 0:1],
                    embT[:, g * 128:(g + 1) * 128].rearrange("p n -> p n ()"))

            # S (edge->node one-hot) from dstS column
            for et in range(NT):
                g = et // TG
                dloc = ph0s.tile([128, 1], F32, tag="dloc")
                nc.vector.tensor_copy(dloc[:], idxs[:, 2 * NT + et:2 * NT + et + 1])
                nc.vector.tensor_scalar_add(dloc[:], dloc[:], float(-128 * g))
                nc.vector.tensor_tensor(S_all[:, et * 128:(et + 1) * 128],
                                        dloc[:].to_broadcast([128, 128]), iota_f[:],
                                        op=ALU.is_equal)

            # distT = exp(G*(d - off)^2), [120, 5*EP]
            dbc = ph0.tile([120, EP], F32, tag="dbc")
            nc.sync.dma_start(dbc[:], d_row_d[0:1, :].to_broadcast([120, EP]))
            distF = ph0.tile([120, 5 * EP], F32, tag="distF")
            for ci in range(5):
                nc.vector.tensor_scalar(distF[:, ci * EP:(ci + 1) * EP], dbc[:],
                                        offc[:, ci:ci + 1], None, op0=ALU.subtract)
            nc.scalar.activation(distF[:], distF[:], AF.Square)
            nc.scalar.activation(distT[:], distF[:], AF.Exp, scale=GCOEF)

        # ---------------- phase A: edge-degree embedding ----------------
        skip_deg = "edgedeg" in ablate
        with tc.tile_pool(name="phA", bufs=1) as phA, \
             tc.tile_pool(name="phAs", bufs=2) as phAs:
            Yt_all = phA.tile([128, NT * NC49], BF, tag="Yt_all")
            nc.sync.dma_start(Yt_all[:], Yt_d[:])
            s2_all = phA.tile([128, EP], BF, tag="s2a")
            radD = phA.tile([128, NT * 896], BF, tag="radD")
            with tc.tile_pool(name="phAp", bufs=1, space="PSUM") as phAp:
                for o, w in ([] if skip_deg else _chunks(EP, 512)):
                    ps = phAp.tile([128, 512], F32, tag="mlp1", space="PSUM", bufs=2)
                    for ci in range(5):
                        nc.tensor.matmul(ps[:, 0:w], lhsT=dW1(ci),
                                         rhs=distT[:, ci * EP + o:ci * EP + o + w],
                                         start=(ci == 0), stop=(ci == 4))
                    s1 = phAs.tile([128, 512], BF, tag="s1")
                    nc.scalar.activation(s1[:, 0:w], ps[:, 0:w], AF.Silu, bias=degb1)
                    ps2 = phAp.tile([128, 512], F32, tag="mlp2", space="PSUM", bufs=2)
                    nc.tensor.matmul(ps2[:, 0:w], lhsT=degw2, rhs=s1[:, 0:w],
                                     start=True, stop=True)
                    nc.scalar.activation(s2_all[:, o:o + w], ps2[:, 0:w], AF.Silu, bias=degb2)
                for et in ([] if skip_deg else range(NT)):
                    ps3 = phAp.tile([128, 896], F32, tag="mlp3", space="PSUM", bufs=2)
                    for o, s in _chunks(7 * C):
                        nc.tensor.matmul(ps3[:, o:o + s],
                                         lhsT=s2_all[:, et * 128:(et + 1) * 128],
                                         rhs=degw3[:, o:o + s], start=True, stop=True)
                    copy_eng(et, radD[:, et * 896:(et + 1) * 896], ps3[:])

            # scatter: PSUM-accumulated over tiles, per 8-coeff chunk
            for g in ([] if skip_deg else range(2)):
                with tc.tile_pool(name=f"degp{g}", bufs=2, space="PSUM") as degp, \
                     tc.tile_pool(name=f"degt{g}", bufs=2, space="PSUM") as degt, \
                     tc.tile_pool(name=f"degs{g}", bufs=2) as degs:
                    for k0 in range(0, NC49, 8):
                        nk = min(8, NC49 - k0)
                        acc = degp.tile([128, 1024], F32, tag="dacc", space="PSUM")
                        for ti in range(TG):
                            et = g * TG + ti
                            M = degs.tile([128, 1024], BF, tag="M")
                            for l in range(L_MAX + 1):
                                ks, kc = LBLK[l]
                                lo, hi = max(ks, k0), min(ks + kc, k0 + nk)
                                if lo >= hi:
                                    continue
                                tt_eng(ti + l).tensor_tensor(
                                    M[:, (lo - k0) * 128:(hi - k0) * 128]
                                        .rearrange("p (k c) -> p k c", c=128),
                                    Yt_all[:, et * NC49 + lo:et * NC49 + hi]
                                        .rearrange("p k -> p k ()").to_broadcast([128, hi - lo, 128]),
                                    radD[:, et * 896 + l * 128:et * 896 + (l + 1) * 128]
                                        .rearrange("p c -> p () c").to_broadcast([128, hi - lo, 128]),
                                    op=ALU.mult)
                            for o, s in _chunks(nk * 128):
                                nc.tensor.matmul(acc[:, o:o + s], lhsT=S_all[:, et * 128:(et + 1) * 128],
                                                 rhs=M[:, o:o + s], start=(ti == 0), stop=(ti == TG - 1))
                        dchunk = degs.tile([128, 1024], BF, tag="dchunk")
                        copy_eng(k0 // 8, dchunk[:, 0:nk * 128], acc[:, 0:nk * 128])
                        tp = degt.tile([128, 1024], BF, tag="dtp", space="PSUM")
                        for j in range(nk):
                            nc.tensor.transpose(tp[:, j * 128:(j + 1) * 128],
                                                dchunk[:, j * 128:(j + 1) * 128], ident_b[:])
                        xs = xT[g][:].rearrange("p (n k) -> p n k", k=NC49)[:, :, k0:k0 + nk]
                        nc.vector.tensor_tensor(
                            xs, xs, tp[:, 0:nk * 128].rearrange("p (j n) -> p n j", j=nk),
                            op=ALU.add)
        with tc.tile_pool(name="phY", bufs=1) as phy, \
             tc.tile_pool(name="phYp", bufs=1, space="PSUM") as phyp:
            sq_t = phy.tile([128, 3136], BF, tag="ph_sq")
            redA = [phy.tile([128, 896], BF, tag=f"ph_red{g}", name=f"ph_red{g}")
                    for g in range(2)]
            stats_pre(0, sq_t, redA[0])
            stats_pre(1, sq_t, redA[1])
            stats_fin("att0", 0, 0, redA[0], phyp, "ph_ms")
            ys_rows(0, 0)
            stats_fin("att0", 0, 1, redA[1], phyp, "ph_ms")
            ys_rows(0, 1)
        dbg("xT0", xT[0][:])
        dbg("xT1", xT[1][:])

        # ---------- attention ----------
        def attention(a, nidx, feed=None):
            last = (a == 2)
            esA = ExitStack()
            ap_ = esA.enter_context(tc.tile_pool(name=f"at{a}", bufs=1))
            log_all = ap_.tile([128, NT * 8], F32, tag="log_all")
            radT = ap_.tile([128, EP], BF, tag="radT")

            # --- radial MLP (norm/y-rows/AG already ran in the prior phase's hook) ---
            with tc.tile_pool(name=f"at{a}r", bufs=2) as rs, \
                 tc.tile_pool(name=f"at{a}rp", bufs=1, space="PSUM") as rp:
                rad_mlp(a, radT, rs, rp)

            # --- edge phase ---
            mp = esA.enter_context(tc.tile_pool(name=f"at{a}m", bufs=1))
            esP = ExitStack()
            pp = esP.enter_context(tc.tile_pool(name=f"at{a}p", bufs=1, space="PSUM"))
            agn = {}

            def alloc_agn(g):
                # last attention keeps both groups' agg; agn1 reuses the ms2
                # rotation (allocated after the final remote gather).
                if last and g == 1:
                    agn[g] = mp.tile([128, W29], BF, tag="ms2", bufs=2, name="agn1")
                elif last:
                    agn[g] = mp.tile([128, W29], BF, tag="agn0", bufs=1, name="agn0")
                else:
                    agn[g] = mp.tile([128, W29], BF, tag="agnX", bufs=1, name=f"agn{g}")

            def feed_pre(g):
                if feed is None:
                    return
                sq_t = mp.tile([128, W29], BF, tag="mtt", bufs=2, name="sq_t")
                red = per.tile([128, 896], BF, tag="redP", bufs=2, name=f"red_{feed}_{g}")
                stats_pre(g, sq_t, red)
                pend_red[(feed, g)] = red

            def group_gather(g):
                vs = [mp.tile([128, W29], BF, tag=f"vsb_{ti}", name=f"vsb{ti}", bufs=1)
                      for ti in range(TG)]
                for ti in range(TG):
                    et = g * TG + ti
                    nc.gpsimd.indirect_dma_start(
                        out=vs[ti][:], out_offset=None, in_=yt_loc[:],
                        in_offset=bass.IndirectOffsetOnAxis(ap=idxs[:, NT + et:NT + et + 1], axis=0))
                return vs

            def group_addrem(g, vs):
                for ti in range(TG):
                    et = g * TG + ti
                    m2 = mp.tile([128, W29], BF, tag="ms2", bufs=2)
                    nc.gpsimd.indirect_dma_start(
                        out=m2[:], out_offset=None, in_=ys_full[:],
                        in_offset=bass.IndirectOffsetOnAxis(ap=idxs[:, et:et + 1], axis=0))
                    nc.vector.tensor_tensor(vs[ti][:], vs[ti][:], m2[:], op=ALU.add)
                if a == 0 and g == 0:
                    dbg("gat0", vs[0][:])

            def tile_T(g, ti, vs):
                et = g * TG + ti
                mt = mp.tile([128, W29], BF, tag="mtt", bufs=2)
                for bi, (r0, nr) in enumerate(RCH8):
                    accT = pp.tile([128, 1024], BF, tag="accT", space="PSUM", bufs=4)
                    for j in range(nr):
                        nc.tensor.transpose(accT[:, j * 128:(j + 1) * 128],
                                            vs[ti][:, (r0 + j) * 128:(r0 + j + 1) * 128],
                                            ident_b[:])
                    nc.vector.tensor_tensor(
                        mt[:, r0 * 128:(r0 + nr) * 128].rearrange("p (r e) -> p r e", e=128),
                        accT[:, 0:nr * 128].rearrange("p (r e) -> p r e", e=128),
                        radT[:, et * 128:(et + 1) * 128].rearrange("p e -> p () e")
                            .to_broadcast([128, nr, 128]),
                        op=ALU.mult)
                if a == 0 and et == 0:
                    dbg("msg00", mt[:])
                return mt

            def tile_V(g, ti, vs, mt):
                et = g * TG + ti
                for bi, (r0, nr) in enumerate(RCH4):
                    accV = pp.tile([128, 512], F32, tag="accV", space="PSUM", bufs=2)
                    for j in range(nr):
                        nc.tensor.matmul(accV[:, j * 128:(j + 1) * 128],
                                         lhsT=mt[:, (r0 + j) * 128:(r0 + j + 1) * 128],
                                         rhs=wV(a), start=True, stop=True)
                    nc.scalar.copy(vs[ti][:, r0 * 128:(r0 + nr) * 128], accV[:, 0:nr * 128])
                qs = mp.tile([128, 128], BF, tag="qs", bufs=2)
                nc.scalar.activation(qs[:], mt[:, 0:128], AF.Silu)
                nc.vector.tensor_scalar(qs[:], qs[:], avecC[:, a:a + 1], None, op0=ALU.mult)
                sx = pp.tile([128, 512], F32, tag="sx", space="PSUM", bufs=2)
                nc.tensor.matmul(sx[:, 0:8], lhsT=qs[:], rhs=Hsel, start=True, stop=True)
                nc.scalar.copy(log_all[:, et * 8:(et + 1) * 8], sx[:, 0:8])
                if a == 0 and et == 0:
                    dbg("vsb00", vs[0][:])

            def group_TV(g, vs):
                mt_prev = tile_T(g, 0, vs)
                for ti in range(1, TG):
                    mt = tile_T(g, ti, vs)
                    tile_V(g, ti - 1, vs, mt_prev)
                    mt_prev = mt
                tile_V(g, TG - 1, vs, mt_prev)

            def softmax(g):
                # logits are bounded (|logit| <~ 24): exact softmax without the
                # max shift — alpha = exp(l)/sum exp(l) is shift-invariant.
                lsl = log_all[:, g * TG * 8:(g + 1) * TG * 8]
                exs = mp.tile([128, TG * 8], BF, tag="exs", bufs=2)
                nc.scalar.activation(exs[:], lsl, AF.Exp)
                return exs

            def sh8_build(g, exs):
                sh = [mp.tile([128, 1024], BF, tag=f"sh8_{ti}", name=f"sh8{ti}", bufs=1)
                      for ti in range(TG)]
                for ti in range(TG):
                    et = g * TG + ti
                    nc.gpsimd.tensor_tensor(
                        sh[ti][:].rearrange("p (h n) -> p h n", h=8),
                        S_all[:, et * 128:(et + 1) * 128].rearrange("p n -> p () n")
                            .to_broadcast([128, 8, 128]),
                        exs[:, ti * 8:(ti + 1) * 8].rearrange("p h -> p h ()")
                            .to_broadcast([128, 8, 128]),
                        op=ALU.mult)
                return sh

            def scatter(g, exs, vs, sh):
                alloc_agn(g)
                dps = pp.tile([128, 512], F32, tag="sx", space="PSUM", bufs=2)
                for ti in range(TG):
                    et = g * TG + ti
                    nc.tensor.matmul(dps[:, 0:8], lhsT=S_all[:, et * 128:(et + 1) * 128],
                                     rhs=exs[:, ti * 8:(ti + 1) * 8],
                                     start=(ti == 0), stop=(ti == TG - 1))
                rden = mp.tile([128, 8], F32, tag="rden", bufs=2)
                nc.vector.tensor_scalar_max(rden[:], dps[:, 0:8], 1e-9)
                nc.vector.reciprocal(rden[:], rden[:])
                agv = agn[g][:].rearrange("p (r h d) -> p h r d", h=8, d=16)
                for h2 in range(HEADS):
                    shacc = pp.tile([128, 512], F32, tag="sx", space="PSUM", bufs=2)
                    for ti in range(TG):
                        nc.tensor.matmul(
                            shacc[:, 0:NR * VPH],
                            lhsT=sh[ti][:, h2 * 128:(h2 + 1) * 128],
                            rhs=vs[ti][:].rearrange("p (r h d) -> p h r d", h=8, d=16)[:, h2],
                            start=(ti == 0), stop=(ti == TG - 1))
                    if h2 % 2 == 0:
                        nc.vector.tensor_scalar(agv[:, h2],
                                                shacc[:, 0:NR * VPH].rearrange("p (r d) -> p r d", d=16),
                                                rden[:, h2:h2 + 1], None, op0=ALU.mult)
                    else:
                        nc.scalar.activation(agv[:, h2],
                                             shacc[:, 0:NR * VPH].rearrange("p (r d) -> p r d", d=16),
                                             AF.Copy, scale=rden[:, h2:h2 + 1])
                if a == 0 and g == 0:
                    dbg("agg00", agn[0][:])

            def project(g):
                ag = mp.tile([128, W29], BF, tag="mtt", bufs=2)
                for bi, (r0, nr) in enumerate(RCH8):
                    acc = pp.tile([128, 1024], BF, tag="accT", space="PSUM", bufs=4)
                    for j in range(nr):
                        nc.tensor.transpose(acc[:, j * 128:(j + 1) * 128],
                                            agn[g][:, (r0 + j) * 128:(r0 + j + 1) * 128],
                                            ident_b[:])
                    copy_eng(bi, ag[:, r0 * 128:(r0 + nr) * 128], acc[:, 0:nr * 128])
                for ci, (o, s) in enumerate(_chunks(W29)):
                    wacc = pp.tile([128, 512], F32, tag="sx", space="PSUM", bufs=2)
                    nc.tensor.matmul(wacc[:, 0:s], lhsT=wP(a), rhs=ag[:, o:o + s],
                                     start=True, stop=True)
                    r0, r1 = o // 128, (o + s) // 128
                    for (os_, ks, cnt) in RBLK:
                        lo, hi = max(os_, r0), min(os_ + cnt, r1)
                        if lo >= hi:
                            continue
                        xv = xT[g][:].rearrange("p (n k) -> p n k", k=NC49)[
                            :, :, ks + (lo - os_):ks + (hi - os_)]
                        nc.vector.tensor_tensor(
                            xv, xv,
                            wacc[:, 0:s].rearrange("p (r n) -> p n r", n=128)[
                                :, :, lo - r0:hi - r0],
                            op=ALU.add)

            # ---- group pipeline ----
            vs0 = group_gather(0)
            group_addrem(0, vs0)
            group_TV(0, vs0)
            exs0 = softmax(0)
            sh0 = sh8_build(0, exs0)
            scatter(0, exs0, vs0, sh0)
            if not last:
                project(0)
            feed_pre(0)
            vs1 = group_gather(1)
            group_addrem(1, vs1)
            group_TV(1, vs1)
            exs1 = softmax(1)
            sh1 = sh8_build(1, exs1)
            scatter(1, exs1, vs1, sh1)
            if not last:
                project(1)
            feed_pre(1)
            if a == 0:
                dbg("logits0", log_all[:])

            if last:
                # pool-first epilogue: pooled[C, (r G)] = w_p^T @ (PT^T @ agn)^T
                esP.close()
                with tc.tile_pool(name="poolEp", bufs=1, space="PSUM") as pep:
                    p2 = pep.tile([16, W29], F32, tag="p2", space="PSUM")
                    for o, s in _chunks(W29):
                        for g in range(2):
                            nc.tensor.matmul(p2[:, o:o + s], lhsT=PT[g][:],
                                             rhs=agn[g][:, o:o + s],
                                             start=(g == 0), stop=(g == 1))
                    p2sb = mp.tile([16, W29], BF, tag="mtt", name="p2sb", bufs=2)
                    nc.vector.tensor_copy(p2sb[:], p2[:])
                with tc.tile_pool(name="poolFp", bufs=1, space="PSUM") as pfp:
                    ptp = pfp.tile([128, NR * G], BF, tag="ptp", space="PSUM")
                    for r in range(NR):
                        nc.tensor.transpose(ptp[:, r * G:(r + 1) * G],
                                            p2sb[:, r * 128:(r + 1) * 128],
                                            ident_b[0:16, 0:16])
                    p2T = mp.tile([128, NR * G], BF, tag="sh8_0", name="p2T", bufs=1)
                    nc.scalar.copy(p2T[:], ptp[:])
                    pps = pfp.tile([128, 512], F32, tag="pps", space="PSUM")
                    nc.tensor.matmul(pps[:, 0:NR * G], lhsT=wP(2), rhs=p2T[:],
                                     start=True, stop=True)
                    pooled_sb = mp.tile([128, NR * G], F32, tag="sh8_1", name="pooled_sb", bufs=1)
                    nc.scalar.copy(pooled_sb[:], pps[:, 0:NR * G])
                    nc.sync.dma_start(pooled_d[:], pooled_sb[:])
                esA.close()
            else:
                esP.close()
                esA.close()

        # ---------- ffn ----------
        def ffn(i, nidx, nxt_a, nxt_nidx):
            key = f"ffn{i}"
            QW = 32 * NC49      # 1568 cols per quarter
            with tc.tile_pool(name=f"ff{i}", bufs=1) as fp:
                hfull = [fp.tile([128, W49], BF, tag=f"hf{g}", name=f"hf{g}") for g in range(2)]
                with tc.tile_pool(name=f"ff{i}fp", bufs=1, space="PSUM") as pfin:
                    for g in range(2):
                        stats_fin(key, nidx, g, pend_red.pop((key, g)), pfin, "ffms")
                for g in range(2):
                    rms_apply(False, hfull[g], g, pend_scl.pop((key, g)))
                sqN = fp.tile([128, 3136], BF, tag="ff_sq")
                redN = [fp.tile([128, 896], BF, tag=f"ff_red{g}", name=f"ff_red{g}")
                        for g in range(2)]
                for g in range(2):
                    with tc.tile_pool(name=f"ff{i}p{g}", bufs=1, space="PSUM") as ffp, \
                         tc.tile_pool(name=f"ff{i}s{g}", bufs=2) as fs:
                        hv = hfull[g][:].rearrange("p (n k) -> p n k", k=NC49)
                        xv = xT[g][:].rearrange("p (n k) -> p n k", k=NC49)
                        for q8 in range(8):
                            n0 = q8 * 16
                            sgs = [fs.tile([128, 16], BF, tag=f"sg{fc}", name=f"sg{fc}",
                                           bufs=1) for fc in range(4)]
                            for klo, khi in ((0, 24), (24, 49)):
                                kn = khi - klo
                                hw_ = 16 * kn
                                ops = ffp.tile([128, 512], F32, tag="ops",
                                               space="PSUM", bufs=2)
                                h1gs = []

                                def mm1(fc):
                                    h1p = ffp.tile([128, 512], F32, tag="h1p",
                                                   space="PSUM", bufs=2)
                                    nc.tensor.matmul(
                                        h1p[:, 0:hw_], lhsT=fW1(i, fc),
                                        rhs=hv[:, n0:n0 + 16, klo:khi],
                                        start=True, stop=True)
                                    return h1p

                                def gate(fc, h1p):
                                    if klo == 0:
                                        nc.scalar.activation(
                                            sgs[fc][:],
                                            h1p[:, 0:hw_].rearrange("p (n k) -> p n k", k=kn)[:, :, 0],
                                            AF.Sigmoid)
                                    h1g = fs.tile([128, 512], BF, tag="h1g")
                                    h1gs.append(h1g)
                                    eng = nc.vector if fc % 2 == 0 else nc.gpsimd
                                    if fc % 2 == 0:
                                        src_ap = h1p[:, 0:hw_].rearrange("p (n k) -> p n k", k=kn)
                                    else:
                                        h1c = fs.tile([128, 512], BF, tag="h1c")
                                        nc.scalar.copy(h1c[:, 0:hw_], h1p[:, 0:hw_])
                                        src_ap = h1c[:, 0:hw_].rearrange("p (n k) -> p n k", k=kn)
                                    eng.tensor_tensor(
                                        h1g[:, 0:hw_].rearrange("p (n k) -> p n k", k=kn),
                                        src_ap,
                                        sgs[fc][:].rearrange("p n -> p n ()")
                                            .to_broadcast([128, 16, kn]),
                                        op=ALU.mult)

                                def mm2(fc):
                                    nc.tensor.matmul(ops[:, 0:hw_], lhsT=fW2(i, fc),
                                                     rhs=h1gs[fc][:, 0:hw_],
                                                     start=(fc == 0), stop=(fc == 3))

                                h1p_prev = mm1(0)
                                gate(0, h1p_prev)
                                h1p_prev = mm1(1)
                                mm2(0)
                                gate(1, h1p_prev)
                                h1p_prev = mm1(2)
                                mm2(1)
                                gate(2, h1p_prev)
                                h1p_prev = mm1(3)
                                mm2(2)
                                gate(3, h1p_prev)
                                mm2(3)
                                xs8 = xv[:, n0:n0 + 16, klo:khi]
                                if klo == 0:
                                    nc.vector.tensor_tensor(
                                        xs8, xs8,
                                        ops[:, 0:hw_].rearrange("p (n k) -> p n k", k=kn),
                                        op=ALU.add)
                                else:
                                    oc = fs.tile([128, 512], F32, tag="oc")
                                    nc.scalar.copy(oc[:, 0:hw_], ops[:, 0:hw_])
                                    nc.gpsimd.tensor_tensor(
                                        xs8, xs8,
                                        oc[:, 0:hw_].rearrange("p (n k) -> p n k", k=kn),
                                        op=ALU.add)
                    stats_pre(g, sqN, redN[g])
                with tc.tile_pool(name=f"ff{i}yp", bufs=1, space="PSUM") as pfy:
                    for g in range(2):
                        stats_fin(f"att{nxt_a}", nxt_nidx, g, redN[g], pfy, "ffms")
                        ys_rows(nxt_a, g)

        attention(0, 0, feed="ffn0")
        dbg("xT0_a0", xT[0][:])
        ffn(0, 1, 1, 2)
        dbg("xT0_f0", xT[0][:])
        attention(1, 2, feed="ffn1")
        ffn(1, 3, 2, 4)
        dbg("xT0_l1", xT[0][:])
        dbg("xT1_l1", xT[1][:])
        attention(2, 4)

    nc.compile()
    return nc


_CACHE = {}


def _get_program(meta, debug=()):
    key = (meta["TG"], tuple(n for n, _ in debug), tuple(sorted(ABLATE)))
    if key not in _CACHE:
        _CACHE[key] = build_program(meta, debug, frozenset(ABLATE))
    return _CACHE[key]


DEBUG_OUTS = ()
ABLATE = set()


class _Runner:
    """Caches the jitted shard_map callable for a compiled program."""

    def __init__(self, nc):
        import jax
        from jax.sharding import Mesh, PartitionSpec
        from jax.experimental.shard_map import shard_map
        from concourse.bass2jax import _bass_exec_p, install_neuronx_cc_hook, partition_id_tensor
        install_neuronx_cc_hook()
        self.jax = jax
        pname = nc.partition_id_tensor.name if nc.partition_id_tensor else None
        in_names, out_names, out_avals, zeros = [], [], [], []
        for alloc in nc.m.functions[0].allocations:
            if not isinstance(alloc, mybir.MemoryLocationSet):
                continue
            name = alloc.memorylocations[0].name
            if alloc.kind == "ExternalInput":
                if name != pname:
                    in_names.append(name)
            elif alloc.kind == "ExternalOutput":
                out_names.append(name)
                shp = tuple(alloc.tensor_shape)
                dt = mybir.dt.np(alloc.dtype)
                out_avals.append(jax.core.ShapedArray(shp, dt))
                zeros.append(np.zeros((NCORES * shp[0],) + shp[1:], dt))
        self.in_names, self.out_names, self.zeros = in_names, out_names, zeros
        n_params, n_outs = len(in_names), len(out_names)
        names_all = in_names + out_names + ([pname] if pname else [])

        def _body(*args):
            operands = list(args)
            if pname is not None:
                operands.append(partition_id_tensor())
            return tuple(_bass_exec_p.bind(
                *operands, out_avals=tuple(out_avals), in_names=tuple(names_all),
                out_names=tuple(out_names), lowering_input_output_aliases=(),
                sim_require_finite=True, sim_require_nnan=True, nc=nc))

        devices = jax.devices()[:NCORES]
        self.mesh = Mesh(np.asarray(devices), ("core",))
        self.fn = jax.jit(shard_map(
            _body, mesh=self.mesh,
            in_specs=(PartitionSpec("core"),) * (n_params + n_outs),
            out_specs=(PartitionSpec("core"),) * n_outs, check_rep=False),
            keep_unused=True)

    def stage(self, in_maps):
        from jax.sharding import NamedSharding, PartitionSpec
        sh = NamedSharding(self.mesh, PartitionSpec("core"))
        args = [np.concatenate([np.asarray(m[n]) for m in in_maps], axis=0)
                for n in self.in_names] + list(self.zeros)
        return [self.jax.device_put(a, sh) for a in args]

    def __call__(self, staged):
        return self.fn(*staged)

    def results(self, outs):
        res = [dict() for _ in range(NCORES)]
        for i, n in enumerate(self.out_names):
            arr = np.asarray(outs[i])
            per = arr.reshape(NCORES, arr.shape[0] // NCORES, *arr.shape[1:])
            for c in range(NCORES):
                res[c][n] = per[c]
        return res


_RUNNERS = {}


def get_runner(meta, debug=()):
    key = (meta["TG"], tuple(n for n, _ in debug))
    if key not in _RUNNERS:
        _RUNNERS[key] = _Runner(_get_program(meta, debug))
    return _RUNNERS[key]


def kernel(**inputs):
    meta, in_maps = host_prep(inputs)
    runner = get_runner(meta, DEBUG_OUTS)
    staged = runner.stage(in_maps)
    runner(staged)          # warmup dispatch
    outs = runner(staged)
    self_results = runner.results(outs)
    # pooled is [C, (r G)] per core; sum cores, then out[g, R[r], c]
    acc = np.zeros((128, NR * G), np.float64)
    for c in range(NCORES):
        acc += self_results[c]["pooled"].astype(np.float64)
    acc = acc.reshape(128, NR, G)            # [c, r, g]
    out = np.zeros((G, NC49, C), np.float32)
    out[:, RESTRICT_NP, :] = acc.transpose(2, 1, 0).astype(np.float32)
    kernel.last_results = self_results
    kernel.last_runner = runner
    kernel.last_staged = staged
    return out.reshape(1, -1)


# revision 51
# speedup vs baseline: 1.0890x; 1.0303x over previous
"""EquiformerV2 (2-layer) Bass/Tile kernel for 8 trn2 NeuronCores — v3.

Sharding: dst-node-range parallel (core c owns nodes [256c, 256c+256) and the
edges terminating there). Per attention: y_s/y_t computed locally in bf16, one
AllGather of y_s, then per 128-edge tile gathered messages feed transposes,
values, logits and a one-hot-matmul scatter.

v3 vs v2: packed weight loads (2 DMAs), radial MLPs batched to 512-col
streams with bias folded into the activation, rad modulation fused into the
transpose-PSUM drain (h-major rad), software-pipelined edge loop (T(i+1)
issued before V(i)), engine-partitioned drains (vector=T, scalar=V), FFN gate
reads PSUM directly, Rsqrt-based norms, pool-first latent epilogue.
"""
import math
from contextlib import ExitStack

import numpy as np

import concourse.bass as bass
import concourse.bacc as bacc
import concourse.mybir as mybir
import concourse.tile as tile
from concourse.bass_utils import run_bass_kernel_spmd
from concourse.masks import make_identity

F32 = mybir.dt.float32
BF = mybir.dt.bfloat16
I32 = mybir.dt.int32
AF = mybir.ActivationFunctionType
ALU = mybir.AluOpType
AX = mybir.AxisListType
BF_NP = mybir.dt.np(BF)

NCORES = 8
L_MAX, M_MAX = 6, 2
NC49 = (L_MAX + 1) ** 2
C = 128
H = 128
HEADS, VPH = 8, 16
FFN = 512
NB = 600
N, E, G = 2048, 12288, 16
NP = N // NCORES
AVG_DEG = 3.0
CUTOFF = 5.0
DISC_LO, DISC_HI = -3.26267, 3.295396
EPS = 1e-6

LBLK = [(l * l, 2 * l + 1) for l in range(L_MAX + 1)]
RBLK = []
_r = 0
for _l in range(L_MAX + 1):
    _cnt = min(2 * _l + 1, 2 * M_MAX + 1)
    RBLK.append((_r, _l * _l + _l - min(_l, M_MAX), _cnt))
    _r += _cnt
NR = _r                   # 29
W29 = NR * 128
W49 = NC49 * 128
RCH8 = [(r0, min(8, NR - r0)) for r0 in range(0, NR, 8)]
RCH4 = [(r0, min(4, NR - r0)) for r0 in range(0, NR, 4)]

_off_np = np.linspace(0.0, CUTOFF, NB).astype(np.float32)
GCOEF = float(-0.5 / (2.0 * (_off_np[1] - _off_np[0])) ** 2)
_mv_np = np.array([m for l in range(L_MAX + 1) for m in range(-l, l + 1)])
_deg_np = np.array([l for l in range(L_MAX + 1) for m in range(-l, l + 1)])
RESTRICT_NP = np.nonzero(np.abs(_mv_np) <= M_MAX)[0]

# bf16 weight pack layout: (name, cols). [120-row blocks live in rows 0:120.]
PACKB = [
    ("w_st", 3 * 256), ("w_v", 3 * 128), ("w_p", 3 * 128),
    ("rad_w2", 3 * 128), ("Hsel", 8), ("ffn_w1", 2 * FFN), ("ffn_w2", 8 * 128),
    ("degw2", 128), ("degw3", 7 * C), ("degw1c", 5 * 128), ("radw1c", 15 * 128),
]
PBOFF = {}
_o = 0
for _n, _w in PACKB:
    PBOFF[_n] = _o
    _o += _w
PBW = _o
# f32 pack: nwT 35 | avecC 3 | offc 5 (rows 0:120) | degb1 1 | degb2 1 | radb1 3
PFW = 48


def real_sph_harm_np(vec):
    r = np.linalg.norm(vec, axis=-1, keepdims=True)
    u = vec / np.maximum(r, 1e-8)
    x, y, z = u[:, 0], u[:, 1], u[:, 2]
    ct = np.clip(z, -1.0, 1.0)
    st = np.sqrt(np.clip(1.0 - ct * ct, 1e-12, 1.0))
    phi = np.arctan2(y, x)
    P = {(0, 0): np.ones_like(ct)}
    for m in range(1, L_MAX + 1):
        P[(m, m)] = -(2 * m - 1) * st * P[(m - 1, m - 1)]
    for m in range(0, L_MAX):
        P[(m + 1, m)] = (2 * m + 1) * ct * P[(m, m)]
    for m in range(0, L_MAX + 1):
        for l in range(m + 2, L_MAX + 1):
            P[(l, m)] = ((2 * l - 1) * ct * P[(l - 1, m)] - (l + m - 1) * P[(l - 2, m)]) / (l - m)
    cols = []
    for l in range(L_MAX + 1):
        for m in range(-l, l + 1):
            am = abs(m)
            nrm = math.sqrt((2 * l + 1) / (4 * math.pi) * math.factorial(l - am) / math.factorial(l + am))
            if m == 0:
                cols.append(nrm * P[(l, 0)])
            elif m > 0:
                cols.append(math.sqrt(2.0) * nrm * P[(l, m)] * np.cos(m * phi))
            else:
                cols.append(math.sqrt(2.0) * nrm * P[(l, am)] * np.sin(am * phi))
    return np.stack(cols, axis=-1).astype(np.float32)


def host_prep(inputs):
    f = lambda k: np.asarray(inputs[k], np.float32)
    pos = f("pos")
    edge_vec = f("edge_vec")
    edge_index = np.asarray(inputs["edge_index"]).astype(np.int64)
    batch = np.asarray(inputs["batch"]).astype(np.int64)

    src, dst = edge_index[0], edge_index[1]
    d_all = np.linalg.norm(edge_vec, axis=-1).astype(np.float32)
    Y_all = (real_sph_harm_np(edge_vec) / np.float32(AVG_DEG)).astype(np.float32)

    t = np.clip(np.round((pos - DISC_LO) / (DISC_HI - DISC_LO) * 128.0 - 0.5), 0, 127).astype(np.int64)
    et_ = f("embed_table")
    emb = (et_[t[:, 0]] + et_[t[:, 1]] + et_[t[:, 2]]).astype(np.float32)

    core_of = dst // NP
    grp_of = (dst % NP) // 128
    lists = [[np.nonzero((core_of == c) & (grp_of == g))[0] for g in range(2)] for c in range(NCORES)]
    TG = max(1, (max(len(lists[c][g]) for c in range(NCORES) for g in range(2)) + 127) // 128)
    NT = 2 * TG
    EP = NT * 128

    cnt = np.bincount(batch, minlength=G).astype(np.float32)
    inv_cnt = (1.0 / np.maximum(cnt, 1.0)).astype(np.float32)

    # ---- f32 pack ----
    nws = [f("attn_norm_w")[0], f("ffn_norm_w")[0], f("attn_norm_w")[1], f("ffn_norm_w")[1], f("final_norm_w")]
    packF = np.zeros((128, PFW), np.float32)
    packF[:, 0:35] = np.concatenate([w.T for w in nws], axis=1)
    packF[:, 35:38] = np.stack([f("alpha_vec")[0].reshape(-1), f("alpha_vec")[1].reshape(-1),
                                f("lat_alpha").reshape(-1)], axis=1)
    packF[0:120, 38:43] = np.ascontiguousarray(_off_np.reshape(5, 120).T)
    packF[:, 43] = f("deg_b1")
    packF[:, 44] = f("deg_b2")
    packF[:, 45] = f("rad_b1")[0]
    packF[:, 46] = f("rad_b1")[1]
    packF[:, 47] = f("lat_rad_b1")

    # ---- bf16 pack ----
    def stack_lat(key, lat_key):
        return np.concatenate([f(key)[0], f(key)[1], f(lat_key)], axis=1)  # [128, 3*128]

    packB = np.zeros((128, PBW), np.float32)

    def put(name, arr, rows=128):
        o = PBOFF[name]
        packB[0:rows, o:o + arr.shape[1]] = arr

    ws_ = stack_lat("w_src", "lat_w_src")     # [128, 3*128]
    wt_ = stack_lat("w_tgt", "lat_w_tgt")
    wst = np.zeros((128, 3 * 256), np.float32)
    for a_ in range(3):
        wst[:, a_ * 256:a_ * 256 + 128] = ws_[:, a_ * 128:(a_ + 1) * 128]
        wst[:, a_ * 256 + 128:a_ * 256 + 256] = wt_[:, a_ * 128:(a_ + 1) * 128]
    put("w_st", wst)
    put("w_v", stack_lat("w_val", "lat_w_val"))
    put("w_p", stack_lat("w_proj", "lat_w_proj"))
    put("rad_w2", stack_lat("rad_w2", "lat_rad_w2"))
    Hsel = np.zeros((128, HEADS), np.float32)
    Hsel[np.arange(128), np.arange(128) // VPH] = 1.0
    put("Hsel", Hsel)
    put("ffn_w1", np.concatenate([f("ffn_w1")[0], f("ffn_w1")[1]], axis=1))
    w2 = np.concatenate([f("ffn_w2")[0], f("ffn_w2")[1]], axis=0)  # [1024, 128]
    put("ffn_w2", w2.reshape(8, 128, 128).transpose(1, 0, 2).reshape(128, 8 * 128))
    put("degw2", f("deg_w2"))
    put("degw3", f("deg_w3"))
    put("degw1c", f("deg_w1").reshape(5, 120, C).transpose(1, 0, 2).reshape(120, 5 * C), rows=120)
    rw1 = np.stack([f("rad_w1")[0], f("rad_w1")[1], f("lat_rad_w1")], axis=0)  # [3, 600, H]
    put("radw1c", rw1.reshape(3, 5, 120, H).transpose(2, 0, 1, 3).reshape(120, 15 * H), rows=120)

    shared = {"packF": packF, "packB": packB.astype(BF_NP)}

    in_maps = []
    for c in range(NCORES):
        srcg = np.zeros((EP,), np.int64)
        dstg = np.zeros((EP,), np.int64)
        dstS = np.full((EP,), 30000, np.int64)
        d_row = np.zeros((1, EP), np.float32)
        Yc = np.zeros((EP, NC49), np.float32)
        for g in range(2):
            idx = lists[c][g]
            o = g * TG * 128
            n = len(idx)
            srcg[o:o + n] = src[idx]
            dstg[o:o + n] = dst[idx] - c * NP
            dstS[o:o + n] = dst[idx] - c * NP
            d_row[0, o:o + n] = d_all[idx]
            Yc[o:o + n] = Y_all[idx]
        embT = np.ascontiguousarray(emb[c * NP:(c + 1) * NP].T)
        PT = np.zeros((NP, G), np.float32)
        nloc = np.arange(c * NP, (c + 1) * NP)
        PT[np.arange(NP), batch[nloc]] = inv_cnt[batch[nloc]]
        YtT = np.ascontiguousarray(Yc.reshape(NT, 128, NC49).transpose(1, 0, 2).reshape(128, NT * NC49))
        # ys_full row layout for split AG: rows [0:1024) = all cores' group-0
        # nodes (row = core*128 + n%128), rows [1024:2048) = group-1 nodes.
        srcr = (srcg // NP) * 128 + (srcg % 128) + 1024 * ((srcg % NP) // 128)
        idxs = np.zeros((128, 3 * NT), np.int32)
        idxs[:, 0:NT] = srcr.reshape(NT, 128).T
        idxs[:, NT:2 * NT] = dstg.reshape(NT, 128).T
        idxs[:, 2 * NT:3 * NT] = dstS.reshape(NT, 128).T
        m = dict(shared)
        m.update({"embT": embT, "d_row": d_row, "Yt": YtT.astype(BF_NP),
                  "idxs": idxs, "PT": PT.astype(BF_NP)})
        in_maps.append(m)
    return {"TG": TG, "NT": NT, "EP": EP}, in_maps


def _chunks(total, step=512):
    o = 0
    while o < total:
        yield o, min(step, total - o)
        o += step


def build_program(meta, debug=(), ablate=frozenset()):
    TG, NT, EP = meta["TG"], meta["NT"], meta["EP"]
    nc = bacc.Bacc("TRN2", target_bir_lowering=False, debug=False, num_devices=NCORES)

    def din(name, shape, dt=F32):
        return nc.dram_tensor(name, shape, dt, kind="ExternalInput")

    packF_d = din("packF", [128, PFW])
    packB_d = din("packB", [128, PBW], BF)
    embT_d = din("embT", [128, NP])
    d_row_d = din("d_row", [1, EP])
    Yt_d = din("Yt", [128, NT * NC49], BF)
    idxs_d = din("idxs", [128, 3 * NT], I32)
    PT_d = din("PT", [NP, G], BF)

    pooled_d = nc.dram_tensor("pooled", [128, NR * G], F32, kind="ExternalOutput")
    dbg_d = {name: nc.dram_tensor("dbg_" + name, list(shape), F32, kind="ExternalOutput")
             for name, shape in debug}

    ys_loc = nc.dram_tensor("ys_loc", [NP, W29], BF)
    yt_loc = nc.dram_tensor("yt_loc", [NP, W29], BF)
    ys_full = nc.dram_tensor("ys_full", [N, W29], BF, addr_space="Shared")
    RG = [list(range(NCORES))]

    with tile.TileContext(nc) as tc, ExitStack() as es:
        per = es.enter_context(tc.tile_pool(name="persist", bufs=1))

        def dbg(name, ap):
            if name in dbg_d:
                if ap.dtype != F32:
                    nc.gpsimd.dma_start(dbg_d[name][:], ap)
                else:
                    nc.sync.dma_start(dbg_d[name][:], ap)

        # ---- persistent tiles ----
        PB = per.tile([128, PBW], BF, tag="PB")
        nc.sync.dma_start(PB[:], packB_d[:])
        PF = per.tile([128, PFW], F32, tag="PF")
        nc.scalar.dma_start(PF[:], packF_d[:])
        idxs = per.tile([128, 3 * NT], I32, tag="idxs")
        nc.gpsimd.dma_start(idxs[:], idxs_d[:])
        PT = [per.tile([128, G], BF, tag=f"PT{g}", name=f"PT{g}") for g in range(2)]
        for g in range(2):
            nc.scalar.dma_start(PT[g][:], PT_d[g * 128:(g + 1) * 128, :])

        def wST(a):
            return PB[:, PBOFF["w_st"] + a * 256:PBOFF["w_st"] + (a + 1) * 256]

        def wV(a):
            return PB[:, PBOFF["w_v"] + a * 128:PBOFF["w_v"] + (a + 1) * 128]

        def wP(a):
            return PB[:, PBOFF["w_p"] + a * 128:PBOFF["w_p"] + (a + 1) * 128]

        def rW2(a):
            return PB[:, PBOFF["rad_w2"] + a * 128:PBOFF["rad_w2"] + (a + 1) * 128]

        def rW1(a, ci):
            o = PBOFF["radw1c"] + (a * 5 + ci) * 128
            return PB[0:120, o:o + 128]

        def fW1(i, fc):
            o = PBOFF["ffn_w1"] + i * FFN + fc * 128
            return PB[:, o:o + 128]

        def fW2(i, fc):
            o = PBOFF["ffn_w2"] + (i * 4 + fc) * 128
            return PB[:, o:o + 128]

        Hsel = PB[:, PBOFF["Hsel"]:PBOFF["Hsel"] + HEADS]
        degw2 = PB[:, PBOFF["degw2"]:PBOFF["degw2"] + 128]
        degw3 = PB[:, PBOFF["degw3"]:PBOFF["degw3"] + 7 * C]

        def dW1(ci):
            o = PBOFF["degw1c"] + ci * 128
            return PB[0:120, o:o + 128]

        nwT = PF[:, 0:35]
        avecC = PF[:, 35:38]
        offc = PF[0:120, 38:43]
        degb1 = PF[:, 43:44]
        degb2 = PF[:, 44:45]

        def radb1(a):
            return PF[:, 45 + a:46 + a]

        ident = per.tile([128, 128], F32, tag="ident")
        make_identity(nc, ident[:])
        ident_b = per.tile([128, 128], BF, tag="identb")
        nc.vector.tensor_copy(ident_b[:], ident[:])
        ones1f = per.tile([1, 128], F32, tag="ones1f")
        nc.vector.memset(ones1f[:], 1.0)
        ones128b = per.tile([128, 128], BF, tag="ones128")
        nc.vector.memset(ones128b[:], 1.0)
        epsc = per.tile([128, 1], F32, tag="epsc")
        nc.vector.memset(epsc[:], EPS)

        xT = [per.tile([128, W49], F32, tag=f"xT{g}", name=f"xT{g}") for g in range(2)]
        S_all = per.tile([128, NT * 128], BF, tag="S_all")
        distT = per.tile([120, 5 * EP], BF, tag="distT")

        copy_rr = [nc.scalar, nc.vector]

        def copy_eng(i, out_ap, in_ap):
            e = copy_rr[i % len(copy_rr)]
            if e is nc.scalar:
                e.copy(out_ap, in_ap)
            else:
                e.tensor_copy(out_ap, in_ap)

        def tt_eng(i):
            return [nc.vector, nc.gpsimd][i % 2]

        # ---------- rms norm split: stats-pre (sq/red), stats-fin (ms/rsqrt) ----
        def stats_pre(g, sq_t, red):
            """red[c,(l n)] = bf16 partials of sum_k x^2; sq_t >=3136-col scratch."""
            QN = 1568
            with nc.allow_low_precision(reason="bf16 ms-reduce, 0.4% on rms"):
                for qi in range(4):
                    sqq = sq_t[:, (qi % 2) * QN:(qi % 2 + 1) * QN]
                    xq = xT[g][:, qi * QN:(qi + 1) * QN]
                    [nc.vector, nc.gpsimd][qi % 2].tensor_tensor(sqq, xq, xq, op=ALU.mult)
                    for l in range(L_MAX + 1):
                        ks, kc = LBLK[l]
                        nc.vector.tensor_reduce(
                            red[:, l * 128 + qi * 32:l * 128 + (qi + 1) * 32],
                            sqq.rearrange("p (n k) -> p n k", k=NC49)[:, :, ks:ks + kc],
                            axis=AX.X, op=ALU.add)

        def stats_fin(key, nidx, g, red, psp, psum_tag):
            scl = per.tile([128, 896], F32, tag="scl", bufs=2, name=f"scl_{key}_{g}")
            msA = psp.tile([128, 512], F32, tag=psum_tag, space="PSUM", bufs=2)
            nc.tensor.matmul(msA[:, 0:512], lhsT=ones128b[:], rhs=red[:, 0:512],
                             start=True, stop=True)
            msB = psp.tile([128, 512], F32, tag=psum_tag, space="PSUM", bufs=2)
            nc.tensor.matmul(msB[:, 0:384], lhsT=ones128b[:], rhs=red[:, 512:896],
                             start=True, stop=True)
            for l in range(L_MAX + 1):
                msl = msA[:, l * 128:(l + 1) * 128] if l < 4 else msB[:, (l - 4) * 128:(l - 3) * 128]
                nc.scalar.activation(scl[:, l * 128:(l + 1) * 128], msl, AF.Ln,
                                     bias=epsc[:], scale=float(1.0 / ((2 * l + 1) * C)))
            nc.scalar.activation(scl[:], scl[:], AF.Exp, scale=-0.5)
            for l in range(L_MAX + 1):
                sl = scl[:, l * 128:(l + 1) * 128]
                nc.vector.tensor_scalar(sl, sl, nwT[:, nidx * 7 + l:nidx * 7 + l + 1],
                                        None, op0=ALU.mult)
            pend_scl[(key, g)] = scl

        def rms_apply(restricted, out_tile, g, scl):
            stride = NR if restricted else NC49
            blocks = RBLK if restricted else [(ks, ks, kc) for (ks, kc) in LBLK]
            for l, (os_, ks, cnt) in enumerate(blocks):
                ov = out_tile[:].rearrange("p (n k) -> p n k", k=stride)[:, :, os_:os_ + cnt]
                xv = xT[g][:].rearrange("p (n k) -> p n k", k=NC49)[:, :, ks:ks + cnt]
                iv = scl[:, l * 128:(l + 1) * 128].rearrange("p n -> p n ()") \
                    .to_broadcast([128, 128, cnt])
                tt_eng(l).tensor_tensor(ov, xv, iv, op=ALU.mult)

        pend_scl = {}
        pend_red = {}

        # ---------- combined ys||yt rows for one group: 29 mm + drains + DMAs ----
        def yrows_st(hr, a, g, ypp, yss, dbg_pref=None):
            hv = hr[:].rearrange("p (n k) -> p k n", k=NR)
            ysrow = yss.tile([128, W29], BF, tag="ysrow", bufs=1)
            ytrow = yss.tile([128, W29], BF, tag="ytrow", bufs=1)
            for bi, (r0, nr) in enumerate(RCH4):
                yp = ypp.tile([128, 1024], F32, tag="yp", space="PSUM", bufs=2)
                for j in range(nr):
                    nc.tensor.matmul(yp[:, j * 256:(j + 1) * 256],
                                     lhsT=hv[:, r0 + j, :], rhs=wST(a),
                                     start=True, stop=True)
                copy_eng(bi, ysrow[:, r0 * 128:(r0 + nr) * 128]
                             .rearrange("p (j c) -> p j c", c=128),
                         yp[:].rearrange("p (j c) -> p j c", c=256)[:, 0:nr, 0:128])
                copy_eng(bi + 1, ytrow[:, r0 * 128:(r0 + nr) * 128]
                             .rearrange("p (j c) -> p j c", c=128),
                         yp[:].rearrange("p (j c) -> p j c", c=256)[:, 0:nr, 128:256])
            if dbg_pref:
                dbg("ysr0", ysrow[:])
                dbg("ytr0", ytrow[:])
            nc.sync.dma_start(ys_loc[g * 128:(g + 1) * 128, :], ysrow[:])
            if "ag" not in ablate:
                nc.gpsimd.collective_compute(
                    "AllGather", ALU.bypass, replica_groups=RG,
                    ins=[ys_loc[g * 128:(g + 1) * 128, :]],
                    outs=[ys_full[g * 1024:(g + 1) * 1024, :]])
            nc.scalar.dma_start(yt_loc[g * 128:(g + 1) * 128, :], ytrow[:])

        def ys_rows(a, g):
            """apply norm + ys/yt rows + AG part for group g (stats precomputed)."""
            scl = pend_scl.pop(("att%d" % a, g))
            with tc.tile_pool(name=f"ys{a}g{g}", bufs=1) as sbp:
                hrT = sbp.tile([128, W29], BF, tag="hrT")
                rms_apply(True, hrT, g, scl)
                with tc.tile_pool(name=f"ys{a}g{g}p", bufs=1, space="PSUM") as ypp:
                    yrows_st(hrT, a, g, ypp, sbp,
                             dbg_pref=(a == 0 and g == 0))

        # ---------- radial MLP for attention a (h-major output) ----------
        def rad_mlp(a, radT, rs, rp):
            for o, w in _chunks(EP, 512):
                ps = rp.tile([128, 512], F32, tag="rmlp1", space="PSUM", bufs=2)
                for ci in range(5):
                    nc.tensor.matmul(ps[:, 0:w], lhsT=rW1(a, ci),
                                     rhs=distT[:, ci * EP + o:ci * EP + o + w],
                                     start=(ci == 0), stop=(ci == 4))
                s1 = rs.tile([128, 512], BF, tag="rm_s1")
                nc.scalar.activation(s1[:, 0:w], ps[:, 0:w], AF.Silu, bias=radb1(a))
                ps2 = rp.tile([128, 512], F32, tag="rmlp2", space="PSUM", bufs=2)
                nc.tensor.matmul(ps2[:, 0:w], lhsT=rW2(a), rhs=s1[:, 0:w],
                                 start=True, stop=True)
                nc.vector.tensor_copy(radT[:, o:o + w], ps2[:, 0:w])

        # ---------------- phase 0: iota, S, distT, xT init ----------------
        with tc.tile_pool(name="ph0", bufs=1) as ph0, \
             tc.tile_pool(name="ph0s", bufs=2) as ph0s:
            iota_i = ph0.tile([128, 128], I32, tag="iotai")
            nc.gpsimd.iota(iota_i[:], pattern=[[1, 128]], base=0, channel_multiplier=0)
            iota_f = ph0.tile([128, 128], F32, tag="iotaf")
            nc.vector.tensor_copy(iota_f[:], iota_i[:])
            embT = ph0.tile([128, NP], F32, tag="embT")
            nc.sync.dma_start(embT[:], embT_d[:])
            for g in range(2):
                nc.gpsimd.memset(xT[g][:], 0.0)
                nc.vector.tensor_copy(
                    xT[g][:].rearrange("p (n k) -> p n k", k=NC49)[:, :, 0:1],
                    embT[:, g * 128:(g + 1) * 128].rearrange("p n -> p n ()"))

            # S (edge->node one-hot) from dstS column
            for et in range(NT):
                g = et // TG
                dloc = ph0s.tile([128, 1], F32, tag="dloc")
                nc.vector.tensor_copy(dloc[:], idxs[:, 2 * NT + et:2 * NT + et + 1])
                nc.vector.tensor_scalar_add(dloc[:], dloc[:], float(-128 * g))
                nc.vector.tensor_tensor(S_all[:, et * 128:(et + 1) * 128],
                                        dloc[:].to_broadcast([128, 128]), iota_f[:],
                                        op=ALU.is_equal)

            # distT = exp(G*(d - off)^2), [120, 5*EP]
            dbc = ph0.tile([120, EP], F32, tag="dbc")
            nc.sync.dma_start(dbc[:], d_row_d[0:1, :].to_broadcast([120, EP]))
            distF = ph0.tile([120, 5 * EP], F32, tag="distF")
            for ci in range(5):
                nc.vector.tensor_scalar(distF[:, ci * EP:(ci + 1) * EP], dbc[:],
                                        offc[:, ci:ci + 1], None, op0=ALU.subtract)
            nc.scalar.activation(distF[:], distF[:], AF.Square)
            nc.scalar.activation(distT[:], distF[:], AF.Exp, scale=GCOEF)

        # ---------------- phase A: edge-degree embedding ----------------
        skip_deg = "edgedeg" in ablate
        with tc.tile_pool(name="phA", bufs=1) as phA, \
             tc.tile_pool(name="phAs", bufs=2) as phAs:
            Yt_all = phA.tile([128, NT * NC49], BF, tag="Yt_all")
            nc.sync.dma_start(Yt_all[:], Yt_d[:])
            s2_all = phA.tile([128, EP], BF, tag="s2a")
            radD = phA.tile([128, NT * 896], BF, tag="radD")
            with tc.tile_pool(name="phAp", bufs=1, space="PSUM") as phAp:
                for o, w in ([] if skip_deg else _chunks(EP, 512)):
                    ps = phAp.tile([128, 512], F32, tag="mlp1", space="PSUM", bufs=2)
                    for ci in range(5):
                        nc.tensor.matmul(ps[:, 0:w], lhsT=dW1(ci),
                                         rhs=distT[:, ci * EP + o:ci * EP + o + w],
                                         start=(ci == 0), stop=(ci == 4))
                    s1 = phAs.tile([128, 512], BF, tag="s1")
                    nc.scalar.activation(s1[:, 0:w], ps[:, 0:w], AF.Silu, bias=degb1)
                    ps2 = phAp.tile([128, 512], F32, tag="mlp2", space="PSUM", bufs=2)
                    nc.tensor.matmul(ps2[:, 0:w], lhsT=degw2, rhs=s1[:, 0:w],
                                     start=True, stop=True)
                    nc.scalar.activation(s2_all[:, o:o + w], ps2[:, 0:w], AF.Silu, bias=degb2)
                for et in ([] if skip_deg else range(NT)):
                    ps3 = phAp.tile([128, 896], F32, tag="mlp3", space="PSUM", bufs=2)
                    for o, s in _chunks(7 * C):
                        nc.tensor.matmul(ps3[:, o:o + s],
                                         lhsT=s2_all[:, et * 128:(et + 1) * 128],
                                         rhs=degw3[:, o:o + s], start=True, stop=True)
                    copy_eng(et, radD[:, et * 896:(et + 1) * 896], ps3[:])

            # scatter: PSUM-accumulated over tiles, per 8-coeff chunk
            for g in ([] if skip_deg else range(2)):
                with tc.tile_pool(name=f"degp{g}", bufs=2, space="PSUM") as degp, \
                     tc.tile_pool(name=f"degt{g}", bufs=2, space="PSUM") as degt, \
                     tc.tile_pool(name=f"degs{g}", bufs=2) as degs:
                    for k0 in range(0, NC49, 8):
                        nk = min(8, NC49 - k0)
                        acc = degp.tile([128, 1024], F32, tag="dacc", space="PSUM")
                        for ti in range(TG):
                            et = g * TG + ti
                            M = degs.tile([128, 1024], BF, tag="M")
                            for l in range(L_MAX + 1):
                                ks, kc = LBLK[l]
                                lo, hi = max(ks, k0), min(ks + kc, k0 + nk)
                                if lo >= hi:
                                    continue
                                tt_eng(ti + l).tensor_tensor(
                                    M[:, (lo - k0) * 128:(hi - k0) * 128]
                                        .rearrange("p (k c) -> p k c", c=128),
                                    Yt_all[:, et * NC49 + lo:et * NC49 + hi]
                                        .rearrange("p k -> p k ()").to_broadcast([128, hi - lo, 128]),
                                    radD[:, et * 896 + l * 128:et * 896 + (l + 1) * 128]
                                        .rearrange("p c -> p () c").to_broadcast([128, hi - lo, 128]),
                                    op=ALU.mult)
                            for o, s in _chunks(nk * 128):
                                nc.tensor.matmul(acc[:, o:o + s], lhsT=S_all[:, et * 128:(et + 1) * 128],
                                                 rhs=M[:, o:o + s], start=(ti == 0), stop=(ti == TG - 1))
                        dchunk = degs.tile([128, 1024], BF, tag="dchunk")
                        copy_eng(k0 // 8, dchunk[:, 0:nk * 128], acc[:, 0:nk * 128])
                        tp = degt.tile([128, 1024], BF, tag="dtp", space="PSUM")
                        for j in range(nk):
                            nc.tensor.transpose(tp[:, j * 128:(j + 1) * 128],
                                                dchunk[:, j * 128:(j + 1) * 128], ident_b[:])
                        xs = xT[g][:].rearrange("p (n k) -> p n k", k=NC49)[:, :, k0:k0 + nk]
                        nc.vector.tensor_tensor(
                            xs, xs, tp[:, 0:nk * 128].rearrange("p (j n) -> p n j", j=nk),
                            op=ALU.add)
                with tc.tile_pool(name=f"phY{g}", bufs=1) as phy, \
                     tc.tile_pool(name=f"phYp{g}", bufs=1, space="PSUM") as phyp:
                    sq_t = phy.tile([128, 3136], BF, tag="ph_sq")
                    redA = phy.tile([128, 896], BF, tag="ph_red")
                    stats_pre(g, sq_t, redA)
                    stats_fin("att0", 0, g, redA, phyp, "ph_ms")
                    ys_rows(0, g)
        dbg("xT0", xT[0][:])
        dbg("xT1", xT[1][:])

        # ---------- attention ----------
        def attention(a, nidx, feed=None):
            last = (a == 2)
            esA = ExitStack()
            ap_ = esA.enter_context(tc.tile_pool(name=f"at{a}", bufs=1))
            log_all = ap_.tile([128, NT * 8], F32, tag="log_all")
            radT = ap_.tile([128, EP], BF, tag="radT")

            # --- radial MLP (norm/y-rows/AG already ran in the prior phase's hook) ---
            with tc.tile_pool(name=f"at{a}r", bufs=2) as rs, \
                 tc.tile_pool(name=f"at{a}rp", bufs=1, space="PSUM") as rp:
                rad_mlp(a, radT, rs, rp)

            # --- edge phase ---
            mp = esA.enter_context(tc.tile_pool(name=f"at{a}m", bufs=1))
            esP = ExitStack()
            pp = esP.enter_context(tc.tile_pool(name=f"at{a}p", bufs=1, space="PSUM"))
            agn = {}

            def alloc_agn(g):
                # last attention keeps both groups' agg; agn1 reuses the ms2
                # rotation (allocated after the final remote gather).
                if last and g == 1:
                    agn[g] = mp.tile([128, W29], BF, tag="ms2", bufs=2, name="agn1")
                elif last:
                    agn[g] = mp.tile([128, W29], BF, tag="agn0", bufs=1, name="agn0")
                else:
                    agn[g] = mp.tile([128, W29], BF, tag="agnX", bufs=1, name=f"agn{g}")

            def feed_pre(g):
                if feed is None:
                    return
                sq_t = mp.tile([128, W29], BF, tag="mtt", bufs=2, name="sq_t")
                red = per.tile([128, 896], BF, tag="redP", bufs=2, name=f"red_{feed}_{g}")
                stats_pre(g, sq_t, red)
                pend_red[(feed, g)] = red

            def group_gather(g):
                vs = [mp.tile([128, W29], BF, tag=f"vsb_{ti}", name=f"vsb{ti}", bufs=1)
                      for ti in range(TG)]
                for ti in range(TG):
                    et = g * TG + ti
                    nc.gpsimd.indirect_dma_start(
                        out=vs[ti][:], out_offset=None, in_=yt_loc[:],
                        in_offset=bass.IndirectOffsetOnAxis(ap=idxs[:, NT + et:NT + et + 1], axis=0))
                return vs

            def group_addrem(g, vs):
                for ti in range(TG):
                    et = g * TG + ti
                    m2 = mp.tile([128, W29], BF, tag="ms2", bufs=2)
                    nc.gpsimd.indirect_dma_start(
                        out=m2[:], out_offset=None, in_=ys_full[:],
                        in_offset=bass.IndirectOffsetOnAxis(ap=idxs[:, et:et + 1], axis=0))
                    nc.vector.tensor_tensor(vs[ti][:], vs[ti][:], m2[:], op=ALU.add)
                if a == 0 and g == 0:
                    dbg("gat0", vs[0][:])

            def tile_T(g, ti, vs):
                et = g * TG + ti
                mt = mp.tile([128, W29], BF, tag="mtt", bufs=2)
                for bi, (r0, nr) in enumerate(RCH8):
                    accT = pp.tile([128, 1024], BF, tag="accT", space="PSUM", bufs=4)
                    for j in range(nr):
                        nc.tensor.transpose(accT[:, j * 128:(j + 1) * 128],
                                            vs[ti][:, (r0 + j) * 128:(r0 + j + 1) * 128],
                                            ident_b[:])
                    nc.vector.tensor_tensor(
                        mt[:, r0 * 128:(r0 + nr) * 128].rearrange("p (r e) -> p r e", e=128),
                        accT[:, 0:nr * 128].rearrange("p (r e) -> p r e", e=128),
                        radT[:, et * 128:(et + 1) * 128].rearrange("p e -> p () e")
                            .to_broadcast([128, nr, 128]),
                        op=ALU.mult)
                if a == 0 and et == 0:
                    dbg("msg00", mt[:])
                return mt

            def tile_V(g, ti, vs, mt):
                et = g * TG + ti
                for bi, (r0, nr) in enumerate(RCH4):
                    accV = pp.tile([128, 512], F32, tag="accV", space="PSUM", bufs=2)
                    for j in range(nr):
                        nc.tensor.matmul(accV[:, j * 128:(j + 1) * 128],
                                         lhsT=mt[:, (r0 + j) * 128:(r0 + j + 1) * 128],
                                         rhs=wV(a), start=True, stop=True)
                    nc.scalar.copy(vs[ti][:, r0 * 128:(r0 + nr) * 128], accV[:, 0:nr * 128])
                qs = mp.tile([128, 128], BF, tag="qs", bufs=2)
                nc.scalar.activation(qs[:], mt[:, 0:128], AF.Silu)
                nc.vector.tensor_scalar(qs[:], qs[:], avecC[:, a:a + 1], None, op0=ALU.mult)
                sx = pp.tile([128, 512], F32, tag="sx", space="PSUM", bufs=2)
                nc.tensor.matmul(sx[:, 0:8], lhsT=qs[:], rhs=Hsel, start=True, stop=True)
                nc.scalar.copy(log_all[:, et * 8:(et + 1) * 8], sx[:, 0:8])
                if a == 0 and et == 0:
                    dbg("vsb00", vs[0][:])

            def group_TV(g, vs):
                mt_prev = tile_T(g, 0, vs)
                for ti in range(1, TG):
                    mt = tile_T(g, ti, vs)
                    tile_V(g, ti - 1, vs, mt_prev)
                    mt_prev = mt
                tile_V(g, TG - 1, vs, mt_prev)

            def softmax(g):
                # logits are bounded (|logit| <~ 24): exact softmax without the
                # max shift — alpha = exp(l)/sum exp(l) is shift-invariant.
                lsl = log_all[:, g * TG * 8:(g + 1) * TG * 8]
                exs = mp.tile([128, TG * 8], BF, tag="exs", bufs=2)
                nc.scalar.activation(exs[:], lsl, AF.Exp)
                return exs

            def sh8_build(g, exs):
                sh = [mp.tile([128, 1024], BF, tag=f"sh8_{ti}", name=f"sh8{ti}", bufs=1)
                      for ti in range(TG)]
                for ti in range(TG):
                    et = g * TG + ti
                    nc.gpsimd.tensor_tensor(
                        sh[ti][:].rearrange("p (h n) -> p h n", h=8),
                        S_all[:, et * 128:(et + 1) * 128].rearrange("p n -> p () n")
                            .to_broadcast([128, 8, 128]),
                        exs[:, ti * 8:(ti + 1) * 8].rearrange("p h -> p h ()")
                            .to_broadcast([128, 8, 128]),
                        op=ALU.mult)
                return sh

            def scatter(g, exs, vs, sh):
                alloc_agn(g)
                dps = pp.tile([128, 512], F32, tag="sx", space="PSUM", bufs=2)
                for ti in range(TG):
                    et = g * TG + ti
                    nc.tensor.matmul(dps[:, 0:8], lhsT=S_all[:, et * 128:(et + 1) * 128],
                                     rhs=exs[:, ti * 8:(ti + 1) * 8],
                                     start=(ti == 0), stop=(ti == TG - 1))
                rden = mp.tile([128, 8], F32, tag="rden", bufs=2)
                nc.vector.tensor_scalar_max(rden[:], dps[:, 0:8], 1e-9)
                nc.vector.reciprocal(rden[:], rden[:])
                agv = agn[g][:].rearrange("p (r h d) -> p h r d", h=8, d=16)
                for h2 in range(HEADS):
                    shacc = pp.tile([128, 512], F32, tag="sx", space="PSUM", bufs=2)
                    for ti in range(TG):
                        nc.tensor.matmul(
                            shacc[:, 0:NR * VPH],
                            lhsT=sh[ti][:, h2 * 128:(h2 + 1) * 128],
                            rhs=vs[ti][:].rearrange("p (r h d) -> p h r d", h=8, d=16)[:, h2],
                            start=(ti == 0), stop=(ti == TG - 1))
                    if h2 % 2 == 0:
                        nc.vector.tensor_scalar(agv[:, h2],
                                                shacc[:, 0:NR * VPH].rearrange("p (r d) -> p r d", d=16),
                                                rden[:, h2:h2 + 1], None, op0=ALU.mult)
                    else:
                        nc.scalar.activation(agv[:, h2],
                                             shacc[:, 0:NR * VPH].rearrange("p (r d) -> p r d", d=16),
                                             AF.Copy, scale=rden[:, h2:h2 + 1])
                if a == 0 and g == 0:
                    dbg("agg00", agn[0][:])

            def project(g):
                ag = mp.tile([128, W29], BF, tag="mtt", bufs=2)
                for bi, (r0, nr) in enumerate(RCH8):
                    acc = pp.tile([128, 1024], BF, tag="accT", space="PSUM", bufs=4)
                    for j in range(nr):
                        nc.tensor.transpose(acc[:, j * 128:(j + 1) * 128],
                                            agn[g][:, (r0 + j) * 128:(r0 + j + 1) * 128],
                                            ident_b[:])
                    copy_eng(bi, ag[:, r0 * 128:(r0 + nr) * 128], acc[:, 0:nr * 128])
                for ci, (o, s) in enumerate(_chunks(W29)):
                    wacc = pp.tile([128, 512], F32, tag="sx", space="PSUM", bufs=2)
                    nc.tensor.matmul(wacc[:, 0:s], lhsT=wP(a), rhs=ag[:, o:o + s],
                                     start=True, stop=True)
                    r0, r1 = o // 128, (o + s) // 128
                    for (os_, ks, cnt) in RBLK:
                        lo, hi = max(os_, r0), min(os_ + cnt, r1)
                        if lo >= hi:
                            continue
                        xv = xT[g][:].rearrange("p (n k) -> p n k", k=NC49)[
                            :, :, ks + (lo - os_):ks + (hi - os_)]
                        nc.vector.tensor_tensor(
                            xv, xv,
                            wacc[:, 0:s].rearrange("p (r n) -> p n r", n=128)[
                                :, :, lo - r0:hi - r0],
                            op=ALU.add)

            # ---- group pipeline ----
            vs0 = group_gather(0)
            group_addrem(0, vs0)
            group_TV(0, vs0)
            exs0 = softmax(0)
            sh0 = sh8_build(0, exs0)
            scatter(0, exs0, vs0, sh0)
            if not last:
                project(0)
            feed_pre(0)
            vs1 = group_gather(1)
            group_addrem(1, vs1)
            group_TV(1, vs1)
            exs1 = softmax(1)
            sh1 = sh8_build(1, exs1)
            scatter(1, exs1, vs1, sh1)
            if not last:
                project(1)
            feed_pre(1)
            if a == 0:
                dbg("logits0", log_all[:])

            if last:
                # pool-first epilogue: pooled[C, (r G)] = w_p^T @ (PT^T @ agn)^T
                esP.close()
                with tc.tile_pool(name="poolEp", bufs=1, space="PSUM") as pep:
                    p2 = pep.tile([16, W29], F32, tag="p2", space="PSUM")
                    for o, s in _chunks(W29):
                        for g in range(2):
                            nc.tensor.matmul(p2[:, o:o + s], lhsT=PT[g][:],
                                             rhs=agn[g][:, o:o + s],
                                             start=(g == 0), stop=(g == 1))
                    p2sb = mp.tile([16, W29], BF, tag="mtt", name="p2sb", bufs=2)
                    nc.vector.tensor_copy(p2sb[:], p2[:])
                with tc.tile_pool(name="poolFp", bufs=1, space="PSUM") as pfp:
                    ptp = pfp.tile([128, NR * G], BF, tag="ptp", space="PSUM")
                    for r in range(NR):
                        nc.tensor.transpose(ptp[:, r * G:(r + 1) * G],
                                            p2sb[:, r * 128:(r + 1) * 128],
                                            ident_b[0:16, 0:16])
                    p2T = mp.tile([128, NR * G], BF, tag="sh8_0", name="p2T", bufs=1)
                    nc.scalar.copy(p2T[:], ptp[:])
                    pps = pfp.tile([128, 512], F32, tag="pps", space="PSUM")
                    nc.tensor.matmul(pps[:, 0:NR * G], lhsT=wP(2), rhs=p2T[:],
                                     start=True, stop=True)
                    pooled_sb = mp.tile([128, NR * G], F32, tag="sh8_1", name="pooled_sb", bufs=1)
                    nc.scalar.copy(pooled_sb[:], pps[:, 0:NR * G])
                    nc.sync.dma_start(pooled_d[:], pooled_sb[:])
                esA.close()
            else:
                esP.close()
                esA.close()

        # ---------- ffn ----------
        def ffn(i, nidx, nxt_a, nxt_nidx):
            key = f"ffn{i}"
            QW = 32 * NC49      # 1568 cols per quarter
            with tc.tile_pool(name=f"ff{i}", bufs=1) as fp:
                hfull = [fp.tile([128, W49], BF, tag=f"hf{g}", name=f"hf{g}") for g in range(2)]
                with tc.tile_pool(name=f"ff{i}fp", bufs=1, space="PSUM") as pfin:
                    for g in range(2):
                        stats_fin(key, nidx, g, pend_red.pop((key, g)), pfin, "ffms")
                for g in range(2):
                    rms_apply(False, hfull[g], g, pend_scl.pop((key, g)))
                sqN = fp.tile([128, 3136], BF, tag="ff_sq")
                redN = [fp.tile([128, 896], BF, tag=f"ff_red{g}", name=f"ff_red{g}")
                        for g in range(2)]
                for g in range(2):
                    with tc.tile_pool(name=f"ff{i}p{g}", bufs=1, space="PSUM") as ffp, \
                         tc.tile_pool(name=f"ff{i}s{g}", bufs=2) as fs:
                        hv = hfull[g][:].rearrange("p (n k) -> p n k", k=NC49)
                        xv = xT[g][:].rearrange("p (n k) -> p n k", k=NC49)
                        for q8 in range(8):
                            n0 = q8 * 16
                            sgs = [fs.tile([128, 16], BF, tag=f"sg{fc}", name=f"sg{fc}",
                                           bufs=1) for fc in range(4)]
                            for klo, khi in ((0, 24), (24, 49)):
                                kn = khi - klo
                                hw_ = 16 * kn
                                ops = ffp.tile([128, 512], F32, tag="ops",
                                               space="PSUM", bufs=2)
                                h1gs = []

                                def mm1(fc):
                                    h1p = ffp.tile([128, 512], F32, tag="h1p",
                                                   space="PSUM", bufs=2)
                                    nc.tensor.matmul(
                                        h1p[:, 0:hw_], lhsT=fW1(i, fc),
                                        rhs=hv[:, n0:n0 + 16, klo:khi],
                                        start=True, stop=True)
                                    return h1p

                                def gate(fc, h1p):
                                    if klo == 0:
                                        nc.scalar.activation(
                                            sgs[fc][:],
                                            h1p[:, 0:hw_].rearrange("p (n k) -> p n k", k=kn)[:, :, 0],
                                            AF.Sigmoid)
                                    h1g = fs.tile([128, 512], BF, tag="h1g")
                                    h1gs.append(h1g)
                                    eng = nc.vector if fc % 2 == 0 else nc.gpsimd
                                    if fc % 2 == 0:
                                        src_ap = h1p[:, 0:hw_].rearrange("p (n k) -> p n k", k=kn)
                                    else:
                                        h1c = fs.tile([128, 512], BF, tag="h1c")
                                        nc.scalar.copy(h1c[:, 0:hw_], h1p[:, 0:hw_])
                                        src_ap = h1c[:, 0:hw_].rearrange("p (n k) -> p n k", k=kn)
                                    eng.tensor_tensor(
                                        h1g[:, 0:hw_].rearrange("p (n k) -> p n k", k=kn),
                                        src_ap,
                                        sgs[fc][:].rearrange("p n -> p n ()")
                                            .to_broadcast([128, 16, kn]),
                                        op=ALU.mult)

                                def mm2(fc):
                                    nc.tensor.matmul(ops[:, 0:hw_], lhsT=fW2(i, fc),
                                                     rhs=h1gs[fc][:, 0:hw_],
                                                     start=(fc == 0), stop=(fc == 3))

                                h1p_prev = mm1(0)
                                gate(0, h1p_prev)
                                h1p_prev = mm1(1)
                                mm2(0)
                                gate(1, h1p_prev)
                                h1p_prev = mm1(2)
                                mm2(1)
                                gate(2, h1p_prev)
                                h1p_prev = mm1(3)
                                mm2(2)
                                gate(3, h1p_prev)
                                mm2(3)
                                xs8 = xv[:, n0:n0 + 16, klo:khi]
                                if klo == 0:
                                    nc.vector.tensor_tensor(
                                        xs8, xs8,
                                        ops[:, 0:hw_].rearrange("p (n k) -> p n k", k=kn),
                                        op=ALU.add)
                                else:
                                    oc = fs.tile([128, 512], F32, tag="oc")
                                    nc.scalar.copy(oc[:, 0:hw_], ops[:, 0:hw_])
                                    nc.gpsimd.tensor_tensor(
                                        xs8, xs8,
                                        oc[:, 0:hw_].rearrange("p (n k) -> p n k", k=kn),
                                        op=ALU.add)
                    stats_pre(g, sqN, redN[g])
                    with tc.tile_pool(name=f"ff{i}yp{g}", bufs=1, space="PSUM") as pfy:
                        stats_fin(f"att{nxt_a}", nxt_nidx, g, redN[g], pfy, "ffms")
                        ys_rows(nxt_a, g)

        attention(0, 0, feed="ffn0")
        dbg("xT0_a0", xT[0][:])
        ffn(0, 1, 1, 2)
        dbg("xT0_f0", xT[0][:])
        attention(1, 2, feed="ffn1")
        ffn(1, 3, 2, 4)
        dbg("xT0_l1", xT[0][:])
        dbg("xT1_l1", xT[1][:])
        attention(2, 4)

    nc.compile()
    return nc


_CACHE = {}


def _get_program(meta, debug=()):
    key = (meta["TG"], tuple(n for n, _ in debug), tuple(sorted(ABLATE)))
    if key not in _CACHE:
        _CACHE[key] = build_program(meta, debug, frozenset(ABLATE))
    return _CACHE[key]


DEBUG_OUTS = ()
ABLATE = set()


class _Runner:
    """Caches the jitted shard_map callable for a compiled program."""

    def __init__(self, nc):
        import jax
        from jax.sharding import Mesh, PartitionSpec
        from jax.experimental.shard_map import shard_map
        from concourse.bass2jax import _bass_exec_p, install_neuronx_cc_hook, partition_id_tensor
        install_neuronx_cc_hook()
        self.jax = jax
        pname = nc.partition_id_tensor.name if nc.partition_id_tensor else None
        in_names, out_names, out_avals, zeros = [], [], [], []
        for alloc in nc.m.functions[0].allocations:
            if not isinstance(alloc, mybir.MemoryLocationSet):
                continue
            name = alloc.memorylocations[0].name
            if alloc.kind == "ExternalInput":
                if name != pname:
                    in_names.append(name)
            elif alloc.kind == "ExternalOutput":
                out_names.append(name)
                shp = tuple(alloc.tensor_shape)
                dt = mybir.dt.np(alloc.dtype)
                out_avals.append(jax.core.ShapedArray(shp, dt))
                zeros.append(np.zeros((NCORES * shp[0],) + shp[1:], dt))
        self.in_names, self.out_names, self.zeros = in_names, out_names, zeros
        n_params, n_outs = len(in_names), len(out_names)
        names_all = in_names + out_names + ([pname] if pname else [])

        def _body(*args):
            operands = list(args)
            if pname is not None:
                operands.append(partition_id_tensor())
            return tuple(_bass_exec_p.bind(
                *operands, out_avals=tuple(out_avals), in_names=tuple(names_all),
                out_names=tuple(out_names), lowering_input_output_aliases=(),
                sim_require_finite=True, sim_require_nnan=True, nc=nc))

        devices = jax.devices()[:NCORES]
        self.mesh = Mesh(np.asarray(devices), ("core",))
        self.fn = jax.jit(shard_map(
            _body, mesh=self.mesh,
            in_specs=(PartitionSpec("core"),) * (n_params + n_outs),
            out_specs=(PartitionSpec("core"),) * n_outs, check_rep=False),
            keep_unused=True)

    def stage(self, in_maps):
        from jax.sharding import NamedSharding, PartitionSpec
        sh = NamedSharding(self.mesh, PartitionSpec("core"))
        args = [np.concatenate([np.asarray(m[n]) for m in in_maps], axis=0)
                for n in self.in_names] + list(self.zeros)
        return [self.jax.device_put(a, sh) for a in args]

    def __call__(self, staged):
        return self.fn(*staged)

    def results(self, outs):
        res = [dict() for _ in range(NCORES)]
        for i, n in enumerate(self.out_names):
            arr = np.asarray(outs[i])
            per = arr.reshape(NCORES, arr.shape[0] // NCORES, *arr.shape[1:])
            for c in range(NCORES):
                res[c][n] = per[c]
        return res


_RUNNERS = {}


def get_runner(meta, debug=()):
    key = (meta["TG"], tuple(n for n, _ in debug))
    if key not in _RUNNERS:
        _RUNNERS[key] = _Runner(_get_program(meta, debug))
    return _RUNNERS[key]


def kernel(**inputs):
    meta, in_maps = host_prep(inputs)
    runner = get_runner(meta, DEBUG_OUTS)
    staged = runner.stage(in_maps)
    runner(staged)          # warmup dispatch
    outs = runner(staged)
    self_results = runner.results(outs)
    # pooled is [C, (r G)] per core; sum cores, then out[g, R[r], c]
    acc = np.zeros((128, NR * G), np.float64)
    for c in range(NCORES):
        acc += self_results[c]["pooled"].astype(np.float64)
    acc = acc.reshape(128, NR, G)            # [c, r, g]
    out = np.zeros((G, NC49, C), np.float32)
    out[:, RESTRICT_NP, :] = acc.transpose(2, 1, 0).astype(np.float32)
    kernel.last_results = self_results
    kernel.last_runner = runner
    kernel.last_staged = staged
    return out.reshape(1, -1)
